# revision 26
# baseline (speedup 1.0000x reference)
"""Trainium2 Bass kernel for nn_Block_78280073937290 (moe_routing).

8-core SPMD plan:
- Token-parallel attention; core c owns true 128-token blocks {c, 15-c}
  (causal-balanced).  All per-core variation is input DATA (the program is
  identical on every core).
- fp32r (tf32-precision, full-rate) matmuls on the attention/shared/expert
  paths; true fp32 matmuls for router logits (top-2 selection is
  precision-critical) and integer-valued index builds.
- MoE: router logits AllGathered; routing/slots computed redundantly on all
  cores from identical fp32 logits; each core indirect-gathers the tokens
  routed to its expert from the AllGathered u, runs the expert FFN, AllGathers
  the compact outputs, and combines its own tokens' two expert rows locally.
- aux-loss partials are computed on device; the scalar is assembled on host.
"""

import numpy as np

import concourse.bass as bass
import concourse.mybir as mybir
import concourse.tile as tile
from concourse import bacc
from concourse.bass import IndirectOffsetOnAxis
from concourse.bass_utils import run_bass_kernel_spmd
from concourse.masks import make_identity

F32 = mybir.dt.float32
F32R = mybir.dt.float32r
I32 = mybir.dt.int32
AF = mybir.ActivationFunctionType
OP = mybir.AluOpType
AX = mybir.AxisListType

B, S, D = 1, 2048, 2048
N, MKV, HD = 16, 4, 128
E, K = 8, 2
MI, MS = 1408, 1408
G = 12
ROT = HD // 2          # 64
HALF = ROT // 2        # 32
THETA = 1024.0
CAP = int(B * S * K / E * 1.25)  # 640
EPS = 1e-5
T = B * S              # 2048
NC = 8
TPC = T // NC          # 256
NB = S // 128          # 16
DT = D // 128          # 16
MT = MI // 128         # 11
INV_SQRT_HD = 1.0 / float(np.sqrt(HD))


def true_block(s):
    return s // 2 if s % 2 == 0 else 15 - s // 2


def r32(ap):
    return ap.bitcast(F32R)


def build_program():
    nc = bacc.Bacc("TRN2", target_bir_lowering=False, debug=False, num_devices=NC)

    def inp(name, shape, dtype=F32):
        return nc.dram_tensor(name, shape, dtype, kind="ExternalInput").ap()

    def inp_r(name, shape):
        return nc.dram_tensor(name, shape, F32R, kind="ExternalInput").ap()

    # activations / tables (per-core data)
    x_nat = inp("x_nat", [TPC, D])
    cos2_t = inp("cos2_t", [ROT, TPC])
    ssin2_t = inp("ssin2_t", [ROT, TPC])
    qpos_bc = inp("qpos_bc", [128, TPC])
    kpos_cols = inp("kpos_cols", [128, NB])
    triu = inp("triu", [128, 128])
    slot_iota = inp("slot_iota", [128, CAP])
    p16_col = inp("p16_col", [128, 1])
    onehot_e = inp("onehot_e", [128, E])
    e_iota = inp("e_iota", [128, E])
    own_rows = inp("own_rows", [128, 2], I32)
    ln1_w = inp("ln1_w", [D])
    ln2_w = inp("ln2_w", [D])
    attn_gate = inp_r("attn_gate", [G, N])
    # weights, host pre-laid-out for contiguous per-partition DMA
    w_q = inp_r("w_q", [D, N * HD])              # natural
    w_k = inp_r("w_k", [D, MKV * HD])
    w_v = inp_r("w_v", [D, MKV * HD])
    w_o = inp_r("w_o", [N * HD, D])
    w_router = inp("wr_h", [128, DT, E])       # (p,dt,e) = w_router[dt*128+p, e]
    w_gs = inp_r("w_gs", [D, MS])
    w_us = inp_r("w_us", [D, MS])
    w_ds = inp_r("w_ds", [MS, D])
    wge_h = inp_r("wge_h", [MT, 128, DT, 128])
    wue_h = inp_r("wue_h", [MT, 128, DT, 128])
    wde_h = inp_r("wde_h", [DT, 128, MT, 128])

    y_out = nc.dram_tensor("y_out", [TPC, D], F32, kind="ExternalOutput").ap()
    stats_out = nc.dram_tensor("stats_out", [1, 32], F32, kind="ExternalOutput").ap()
    dbg_logits = nc.dram_tensor("dbg_logits", [T, E], F32, kind="ExternalOutput").ap()

    kT_loc = nc.dram_tensor("kT_loc", [MKV, 128, TPC], F32R).ap()
    kT_ag = nc.dram_tensor("kT_ag", [NC * MKV, 128, TPC], F32R, addr_space="Shared").ap()
    # v stored partition-major per kv head: (kvh, p, tt, hd)
    v_loc = nc.dram_tensor("v_loc", [MKV, 128, 2, HD], F32R).ap()
    v_ag = nc.dram_tensor("v_ag", [NC * MKV, 128, 2, HD], F32R, addr_space="Shared").ap()
    u_loc = nc.dram_tensor("u_loc", [TPC, D], mybir.dt.bfloat16).ap()
    u_ag = nc.dram_tensor("u_ag", [T, D], mybir.dt.bfloat16, addr_space="Shared").ap()
    lg_loc = nc.dram_tensor("lg_loc", [TPC, E], F32).ap()
    lg_ag = nc.dram_tensor("lg_ag", [T, E], F32, addr_space="Shared").ap()
    yb_loc = nc.dram_tensor("yb_loc", [CAP, D], mybir.dt.bfloat16).ap()
    yb_ag = nc.dram_tensor("yb_ag", [E * CAP, D], mybir.dt.bfloat16, addr_space="Shared").ap()
    idx_dram = nc.dram_tensor("idx_dram", [CAP], I32).ap()
    fi_dram = nc.dram_tensor("fi_dram", [T, 4], F32).ap()

    RG = [list(range(NC))]

    with tile.TileContext(nc) as tc:
        with (
            tc.tile_pool(name="persist", bufs=1) as pp,
            tc.tile_pool(name="const", bufs=1) as cp,
        ):
            ident = cp.tile([128, 128], F32, tag="ident")
            make_identity(nc, ident)
            ident_r = cp.tile([128, 128], F32R, tag="ident_r")
            ones_col_r = cp.tile([128, 1], F32R, tag="ones_col_r")
            ones_col = cp.tile([128, 1], F32, tag="ones_col")
            nc.vector.memset(ones_col[:], 1.0)
            ones_row = cp.tile([1, 128], F32, tag="ones_row")
            nc.vector.memset(ones_row[:], 1.0)
            nc.vector.tensor_copy(ident_r[:], ident[:])
            ident_b = cp.tile([128, 128], mybir.dt.bfloat16, tag="ident_b")
            nc.vector.tensor_copy(ident_b[:], ident[:])
            nc.vector.tensor_copy(ones_col_r[:], ones_col[:])
            eps_ln = cp.tile([128, 1], F32, tag="eps_ln")
            nc.vector.memset(eps_ln[:], EPS)
            eps_qk = cp.tile([1, 1], F32, tag="eps_qk")
            nc.vector.memset(eps_qk[:], 1e-6)

            def load_const(ap_dram, shape, tag, dtype=F32):
                t = cp.tile(shape, dtype, tag=tag)
                nc.sync.dma_start(t[:], ap_dram)
                return t

            cos2_sb = load_const(cos2_t, [ROT, TPC], "cos2")
            ssin2_sb = load_const(ssin2_t, [ROT, TPC], "ssin2")
            qpos_sb = load_const(qpos_bc, [128, TPC], "qpos")
            kpos_sb = load_const(kpos_cols, [128, NB], "kpos")
            triu_sb = load_const(triu, [128, 128], "triu")
            slot_iota_sb = load_const(slot_iota, [128, CAP], "slot_iota")
            p16_sb = load_const(p16_col, [128, 1], "p16")
            onehot_sb = load_const(onehot_e, [128, E], "onehot")
            eiota_sb = load_const(e_iota, [128, E], "eiota")
            own_rows_sb = load_const(own_rows, [128, 2], "own_rows", I32)
            ln1_col = load_const(ln1_w.rearrange("(o p) -> p o", p=128), [128, DT], "ln1col")
            ln2_col = load_const(ln2_w.rearrange("(o p) -> p o", p=128), [128, DT], "ln2col")
            ag_sb = load_const(attn_gate, [G, N], "ag", F32R)

            h_sb = pp.tile([128, 2, D], F32, tag="h_sb")

            uTrp = tc.alloc_tile_pool(name="uTrp", bufs=1)
            uTr = uTrp.tile([128, DT, TPC], F32R, tag="uTr")
            uTp = tc.alloc_tile_pool(name="uTp", bufs=1)
            uT = uTp.tile([128, DT, TPC], F32, tag="uT")
            actp = tc.alloc_tile_pool(name="actp", bufs=1)
            u1T = actp.tile([128, DT, TPC], F32R, tag="u1T")
            qT = actp.tile([128, N, TPC], F32R, tag="qT")
            oT = actp.tile([128, N, TPC], F32R, tag="oT")
            gZ = actp.tile([1, N, TPC], F32, tag="gZ")

            # ---------- PHASE 0: u1 = rmsnorm(x); u1T ----------
            with (
                tc.tile_pool(name="ph0", bufs=2) as wp,
                tc.tile_pool(name="ph0ps", bufs=2, space="PSUM") as ps,
            ):
                u1_nat = wp.tile([128, 2, D], F32, tag="u1nat", bufs=1)
                x_ph0 = wp.tile([128, 2, D], F32, tag="x_ph0", bufs=1)
                for tt in range(2):
                    nc.sync.dma_start(x_ph0[:, tt, :], x_nat[tt * 128:(tt + 1) * 128, :])
                for tt in range(2):
                    sq = wp.tile([128, D], F32, tag="sq")
                    nc.vector.tensor_tensor(sq[:], x_ph0[:, tt, :], x_ph0[:, tt, :], op=OP.mult)
                    ssum = wp.tile([128, 1], F32, tag="ssum")
                    nc.vector.tensor_reduce(ssum[:], sq[:], axis=AX.X, op=OP.add)
                    rt = wp.tile([128, 1], F32, tag="rt")
                    nc.scalar.activation(rt[:], ssum[:], AF.Sqrt, scale=1.0 / D, bias=eps_ln[:])
                    nc.vector.reciprocal(rt[:], rt[:])
                    nc.vector.tensor_scalar(u1_nat[:, tt, :], x_ph0[:, tt, :], rt[:], None, op0=OP.mult)
                for dt in range(DT):
                    for tt in range(2):
                        pt = ps.tile([128, 128], F32, tag="tp")
                        nc.tensor.transpose(pt[:], u1_nat[:, tt, dt * 128:(dt + 1) * 128], ident[:])
                        nc.vector.tensor_scalar(u1T[:, dt, tt * 128:(tt + 1) * 128], pt[:],
                                                ln1_col[:, dt:dt + 1], None, op0=OP.mult)

            # ---------- PHASE 1: QKV (natural, N=512 moving), norms, rope ----------
            with (
                tc.tile_pool(name="ph1", bufs=2) as wp,
                tc.tile_pool(name="ph1r", bufs=1) as rp,
                tc.tile_pool(name="ph1w", bufs=2) as wgt,
            ):
                rq_flat = rp.tile([1, N, TPC], F32, tag="rq_flat")
                rk_flat = rp.tile([1, MKV, TPC], F32, tag="rk_flat")
                wk_r = w_k.rearrange("(dt p) m -> dt p m", p=128)
                wv_r = w_v.rearrange("(dt p) m -> dt p m", p=128)
                wq_r = w_q.rearrange("(dt p) m -> dt p m", p=128)

                # --- pass 1: k_nat, v_nat ---
                ps_kv = tc.alloc_tile_pool(name="ps_kv", bufs=1, space="PSUM")
                pk = [ps_kv.tile([128, MKV * HD], F32, tag=f"pk{tt}", name=f"pk{tt}")
                      for tt in range(2)]
                pv = [ps_kv.tile([128, MKV * HD], F32, tag=f"pv{tt}", name=f"pv{tt}")
                      for tt in range(2)]
                for dt in range(DT):
                    wtk = wgt.tile([128, MKV * HD], F32R, tag="wtk")
                    nc.sync.dma_start(wtk[:], wk_r[dt])
                    wtv = wgt.tile([128, MKV * HD], F32R, tag="wtv")
                    nc.sync.dma_start(wtv[:], wv_r[dt])
                    for tt in range(2):
                        nc.tensor.matmul(pk[tt][:], u1T[:, dt, tt * 128:(tt + 1) * 128],
                                         wtk[:], start=(dt == 0), stop=(dt == DT - 1))
                        nc.tensor.matmul(pv[tt][:], u1T[:, dt, tt * 128:(tt + 1) * 128],
                                         wtv[:], start=(dt == 0), stop=(dt == DT - 1))
                k_nat = rp.tile([128, 2, MKV * HD], F32R, tag="k_nat")
                v_sb = rp.tile([128, 2, MKV * HD], F32R, tag="v_sb")
                for tt in range(2):
                    nc.vector.tensor_copy(k_nat[:, tt, :], pk[tt][:])
                    nc.vector.tensor_copy(v_sb[:, tt, :], pv[tt][:])
                ps_kv.release()

                ps_m1 = tc.alloc_tile_pool(name="ps_m1", bufs=1, space="PSUM")
                for kh in range(MKV):
                    for tt in range(2):
                        nc.sync.dma_start(v_loc[kh, :, tt, :], v_sb[:, tt, kh * HD:(kh + 1) * HD])
                nc.gpsimd.collective_compute("AllGather", OP.bypass, replica_groups=RG,
                                             ins=[v_loc[:]], outs=[v_ag[:]])

                def qknorm_rope(dst, rdst, wp=wp, ps1=None):
                    """dst: F32R sbuf [128, TPC] (in place); rdst [1, TPC] @p0."""
                    sq = wp.tile([128, TPC], F32, tag="sqh")
                    nc.vector.tensor_tensor(sq[:], dst, dst, op=OP.mult)
                    sp = ps1.tile([1, TPC], F32, tag="normsum", bufs=1)
                    nc.tensor.matmul(sp[:], ones_col[:], sq[:], start=True, stop=True)
                    nc.scalar.activation(rdst, sp[:], AF.Sqrt, scale=1.0 / HD, bias=eps_qk[:])
                    nc.vector.reciprocal(rdst, rdst)
                    xs = wp.tile([ROT, TPC], F32, tag="xs")
                    nc.sync.dma_start(xs[0:HALF, :], dst[HALF:ROT, :].bitcast(F32))
                    nc.sync.dma_start(xs[HALF:ROT, :], dst[0:HALF, :].bitcast(F32))
                    tmp = wp.tile([ROT, TPC], F32, tag="tmp_rope")
                    nc.vector.tensor_tensor(tmp[:], xs[:], ssin2_sb[:], op=OP.mult)
                    xc = wp.tile([ROT, TPC], F32, tag="xc")
                    nc.vector.tensor_tensor(xc[:], dst[0:ROT, :], cos2_sb[:], op=OP.mult)
                    nc.vector.tensor_tensor(dst[0:ROT, :], xc[:], tmp[:], op=OP.add)

                def fold_scale(dst, row_ap, psx):
                    bp = psx.tile([128, TPC], F32, tag="bcast", bufs=1)
                    nc.tensor.matmul(bp[:], ones_row[:], row_ap, start=True, stop=True)
                    bs = wp.tile([128, TPC], F32, tag="bcast_sb")
                    nc.vector.tensor_copy(bs[:], bp[:])
                    nc.vector.tensor_tensor(dst, dst, bs[:], op=OP.mult)

                kT_sb = rp.tile([128, MKV, TPC], F32R, tag="kT_sb")
                for kh in range(MKV):
                    for tt in range(2):
                        ktp = ps_m1.tile([128, 128], F32R, tag="ktp", bufs=2)
                        nc.tensor.transpose(ktp[:], k_nat[:, tt, kh * 128:(kh + 1) * 128],
                                            ident_r[:])
                        nc.vector.tensor_copy(kT_sb[:, kh, tt * 128:(tt + 1) * 128], ktp[:])
                    qknorm_rope(kT_sb[:, kh, :], rk_flat[:, kh, :], ps1=ps_m1)
                    fold_scale(kT_sb[:, kh, :], rk_flat[:, kh, :], ps_m1)
                    nc.sync.dma_start(kT_loc[kh], kT_sb[:, kh, :])
                nc.gpsimd.collective_compute("AllGather", OP.bypass, replica_groups=RG,
                                             ins=[kT_loc[:]], outs=[kT_ag[:]])
                ps_m1.release()

                # --- pass 2: q_nat ---
                ps_q = tc.alloc_tile_pool(name="ps_q", bufs=1, space="PSUM")
                pq = [[ps_q.tile([128, 512], F32, tag=f"pq{tt}{ch}", name=f"pq{tt}{ch}")
                       for ch in range(4)] for tt in range(2)]
                for dt in range(DT):
                    wtq = wgt.tile([128, N * HD], F32R, tag="wtq")
                    nc.sync.dma_start(wtq[:], wq_r[dt])
                    for tt in range(2):
                        for ch in range(4):
                            nc.tensor.matmul(pq[tt][ch][:],
                                             u1T[:, dt, tt * 128:(tt + 1) * 128],
                                             wtq[:, ch * 512:(ch + 1) * 512],
                                             start=(dt == 0), stop=(dt == DT - 1))
                q_nat = rp.tile([128, 2, N * HD], F32R, tag="q_nat")
                for tt in range(2):
                    for ch in range(4):
                        nc.vector.tensor_copy(q_nat[:, tt, ch * 512:(ch + 1) * 512],
                                              pq[tt][ch][:])
                ps_q.release()

                ps_m2 = tc.alloc_tile_pool(name="ps_m2", bufs=1, space="PSUM")
                for h in range(N):
                    for tt in range(2):
                        qtp = ps_m2.tile([128, 128], F32R, tag="qtp", bufs=2)
                        nc.tensor.transpose(qtp[:], q_nat[:, tt, h * 128:(h + 1) * 128],
                                            ident_r[:])
                        nc.vector.tensor_copy(qT[:, h, tt * 128:(tt + 1) * 128], qtp[:])
                    qknorm_rope(qT[:, h, :], rq_flat[:, h, :], ps1=ps_m2)
                    fold_scale(qT[:, h, :], rq_flat[:, h, :], ps_m2)
                # attention output gate
                for h in range(N):
                    gp = ps_m2.tile([1, TPC], F32, tag="gTp", bufs=1)
                    nc.tensor.matmul(gp[:], ag_sb[:, h:h + 1], u1T[0:G, 0, :],
                                     start=True, stop=True)
                    nc.scalar.activation(gZ[:, h, :], gp[:], AF.Sigmoid)
                nc.vector.tensor_scalar(gZ[:].rearrange("o h t -> o (h t)"),
                                        gZ[:].rearrange("o h t -> o (h t)"),
                                        2.0, None, op0=OP.mult)
                ps_m2.release()

            # ---------- PHASE 2: attention core ----------
            with (
                tc.tile_pool(name="att", bufs=2) as wp,
                tc.tile_pool(name="attkv", bufs=1) as kvp,
                tc.tile_pool(name="attpt", bufs=16) as ptp,
                tc.tile_pool(name="attps_s", bufs=2, space="PSUM") as ps_s,
                tc.tile_pool(name="attps_t", bufs=2, space="PSUM") as ps_t,
                tc.tile_pool(name="attps_o", bufs=2, space="PSUM") as ps_o,
                tc.tile_pool(name="attps_z", bufs=1, space="PSUM") as ps_z,
                tc.tile_pool(name="attps_b", bufs=1, space="PSUM") as ps_b,
            ):
                m01T = kvp.tile([128, NB, TPC], mybir.dt.bfloat16, tag="m01T")
                for s in range(NB):
                    nc.vector.tensor_scalar(m01T[:, s, :], qpos_sb[:], kpos_sb[:, s:s + 1],
                                            None, op0=OP.is_ge)
                for kvh in range(MKV):
                    kT_h = kvp.tile([128, NB * 128], F32R, tag="kT_h")
                    for c in range(NC):
                        nc.sync.dma_start(kT_h[:, c * 256:(c + 1) * 256],
                                          kT_ag[c * MKV + kvh, :, :])
                    v_h = kvp.tile([128, NB, HD], F32R, tag="v_h")
                    for c in range(NC):
                        nc.sync.dma_start(v_h[:, 2 * c:2 * c + 2, :],
                                          v_ag[c * MKV + kvh, :, :, :])
                    kT_h3 = kT_h[:].rearrange("p (s q) -> p s q", q=128)
                    for qh in range(4):
                        h = kvh * 4 + qh
                        probsT = [ptp.tile([128, TPC], F32R, tag="probsT", name=f"probsT{h}_{si}")
                                  for si in range(NB)]
                        for qb in range(2):
                            nslots = 8 if qb == 0 else NB
                            pt_buf = wp.tile([128, NB * 128], F32R, tag="ptbuf", bufs=1)
                            nmm = 2 if qb == 0 else 4
                            for m in range(nmm):
                                sp = ps_s.tile([128, 512], F32, tag="scores")
                                if qb == 0:
                                    rhs = kT_h3[:, ::2, :][:, m * 4:(m + 1) * 4, :]
                                else:
                                    rhs = kT_h[:, m * 512:(m + 1) * 512]
                                nc.tensor.matmul(sp[:], qT[:, h, qb * 128:(qb + 1) * 128],
                                                 rhs, start=True, stop=True)
                                nc.scalar.activation(pt_buf[:, m * 512:(m + 1) * 512], sp[:],
                                                     AF.Exp, scale=INV_SQRT_HD)
                            for i in range(nslots):
                                s = 2 * i if qb == 0 else i
                                tp = ps_t.tile([128, 128], F32R, tag="trp")
                                nc.tensor.transpose(tp[:], pt_buf[:, i * 128:(i + 1) * 128], ident_r[:])
                                nc.vector.tensor_tensor(
                                    probsT[s][:, qb * 128:(qb + 1) * 128], tp[:],
                                    m01T[:, s, qb * 128:(qb + 1) * 128], op=OP.mult)
                        zp = ps_z.tile([1, TPC], F32, tag="z")
                        op_ = ps_o.tile([128, TPC], F32, tag="oTp")
                        for s in range(NB):
                            if s % 2 == 0:
                                rhs, zsl, osl = probsT[s][:], zp[:], op_[:]
                            else:
                                rhs = probsT[s][:, 128:TPC]
                                zsl, osl = zp[:, 128:TPC], op_[:, 128:TPC]
                            first, last = s == 0, s == NB - 1
                            nc.tensor.matmul(zsl, ones_col_r[:], rhs, start=first, stop=last)
                            nc.tensor.matmul(osl, v_h[:, s, :], rhs, start=first, stop=last)
                        srow = wp.tile([1, TPC], F32, tag="srow")
                        nc.vector.reciprocal(srow[:], zp[:])
                        nc.vector.tensor_tensor(srow[:], srow[:], gZ[:, h, :], op=OP.mult)
                        bp = ps_b.tile([128, TPC], F32, tag="sb_bc")
                        nc.tensor.matmul(bp[:], ones_row[:], srow[:], start=True, stop=True)
                        bs = wp.tile([128, TPC], F32, tag="sb_bc_sb")
                        nc.vector.tensor_copy(bs[:], bp[:])
                        nc.vector.tensor_tensor(oT[:, h, :], op_[:], bs[:], op=OP.mult)

            # ---------- PHASE 2b: w_o, residual, ln2, uT ----------
            with (
                tc.tile_pool(name="wo", bufs=2) as wgt,
                tc.tile_pool(name="wops", bufs=1, space="PSUM") as ps,
            ):
                wo_r = w_o.rearrange("(h p) d -> h p d", p=128)
                x_rb = wgt.tile([128, 2, D], F32, tag="x_rb", bufs=1)
                for tt in range(2):
                    nc.sync.dma_start(x_rb[:, tt, :], x_nat[tt * 128:(tt + 1) * 128, :])
                pgrid = [[ps.tile([128, 512], F32, tag=f"wop{qb}{ch}", name=f"wop{qb}{ch}")
                          for ch in range(4)] for qb in range(2)]
                for h in range(N):
                    wo_h = wgt.tile([128, D], F32R, tag="wo_h")
                    nc.sync.dma_start(wo_h[:], wo_r[h])
                    for qb in range(2):
                        for ch in range(4):
                            nc.tensor.matmul(pgrid[qb][ch][:],
                                             oT[:, h, qb * 128:(qb + 1) * 128],
                                             wo_h[:, ch * 512:(ch + 1) * 512],
                                             start=(h == 0), stop=(h == N - 1))
                for qb in range(2):
                    for ch in range(4):
                        nc.vector.tensor_tensor(h_sb[:, qb, ch * 512:(ch + 1) * 512],
                                                pgrid[qb][ch][:],
                                                x_rb[:, qb, ch * 512:(ch + 1) * 512], op=OP.add)
            with (
                tc.tile_pool(name="ln2", bufs=2) as wp,
                tc.tile_pool(name="ln2ps", bufs=2, space="PSUM") as ps,
            ):
                u_sb = wp.tile([128, 2, D], F32, tag="u_sb", bufs=1)
                for tt in range(2):
                    sq = wp.tile([128, D], F32, tag="sq2")
                    nc.vector.tensor_tensor(sq[:], h_sb[:, tt, :], h_sb[:, tt, :], op=OP.mult)
                    ssum = wp.tile([128, 1], F32, tag="ssum2")
                    nc.vector.tensor_reduce(ssum[:], sq[:], axis=AX.X, op=OP.add)
                    rt = wp.tile([128, 1], F32, tag="rt2")
                    nc.scalar.activation(rt[:], ssum[:], AF.Sqrt, scale=1.0 / D, bias=eps_ln[:])
                    nc.vector.reciprocal(rt[:], rt[:])
                    nc.vector.tensor_scalar(u_sb[:, tt, :], h_sb[:, tt, :], rt[:], None, op0=OP.mult)
                    u_bf = wp.tile([128, D], mybir.dt.bfloat16, tag="u_bf")
                    nc.vector.tensor_copy(u_bf[:], u_sb[:, tt, :])
                    nc.sync.dma_start(u_loc[tt * 128:(tt + 1) * 128, :], u_bf[:])
                for dt in range(DT):
                    for tt in range(2):
                        pt = ps.tile([128, 128], F32, tag="tpu")
                        nc.tensor.transpose(pt[:], u_sb[:, tt, dt * 128:(dt + 1) * 128], ident[:])
                        nc.vector.tensor_scalar(uT[:, dt, tt * 128:(tt + 1) * 128], pt[:],
                                                ln2_col[:, dt:dt + 1], None, op0=OP.mult)
                        nc.vector.tensor_scalar(uTr[:, dt, tt * 128:(tt + 1) * 128], pt[:],
                                                ln2_col[:, dt:dt + 1], None, op0=OP.mult)
            actp.release()

            # ---------- PHASE 3: router logits (fp32), AGs ----------
            with (
                tc.tile_pool(name="rt", bufs=2) as wp,
                tc.tile_pool(name="rtps", bufs=2, space="PSUM") as ps,
            ):
                wr_sb = wp.tile([128, DT, E], F32, tag="wr")
                nc.sync.dma_start(wr_sb[:], w_router[:])
                lg_sb = wp.tile([128, 2, E], F32, tag="lg")
                for tt in range(2):
                    lp = ps.tile([128, E], F32, tag="lgp")
                    for dt in range(DT):
                        nc.tensor.matmul(lp[:], uT[:, dt, tt * 128:(tt + 1) * 128],
                                         wr_sb[:, dt, :], start=(dt == 0), stop=(dt == DT - 1))
                    nc.vector.tensor_copy(lg_sb[:, tt, :], lp[:])
                    nc.sync.dma_start(lg_loc[tt * 128:(tt + 1) * 128, :], lg_sb[:, tt, :])
                nc.gpsimd.collective_compute("AllGather", OP.bypass, replica_groups=RG,
                                             ins=[lg_loc[:]], outs=[lg_ag[:]])
                nc.gpsimd.collective_compute("AllGather", OP.bypass, replica_groups=RG,
                                             ins=[u_loc[:]], outs=[u_ag[:]])
                nc.sync.dma_start(dbg_logits[:], lg_ag[:])

            # ---------- shared expert (natural layout, split for AG overlap) ----
            shp = tc.alloc_tile_pool(name="shp", bufs=1)
            hsT = shp.tile([128, MT, TPC], F32R, tag="hsT")
            shared_nat = shp.tile([128, 2, D], F32, tag="shared_nat")
            SCH = [(0, 512), (512, 1024), (1024, MS)]

            def shared_gate_up(tt):
                with (
                    tc.tile_pool(name=f"shA{tt}", bufs=2) as wp,
                    tc.tile_pool(name=f"shAw{tt}", bufs=3) as wgt,
                    tc.tile_pool(name=f"shAps{tt}", bufs=1, space="PSUM") as ps,
                ):
                    wgs_r = w_gs.rearrange("(dt p) m -> dt p m", p=128)
                    wus_r = w_us.rearrange("(dt p) m -> dt p m", p=128)
                    gp = [ps.tile([128, c1 - c0], F32, tag=f"sgp{i}", name=f"sgp{i}")
                          for i, (c0, c1) in enumerate(SCH)]
                    up = [ps.tile([128, c1 - c0], F32, tag=f"sup{i}", name=f"sup{i}")
                          for i, (c0, c1) in enumerate(SCH)]
                    for dt in range(DT):
                        wg = wgt.tile([128, MS], F32R, tag="wgs")
                        nc.sync.dma_start(wg[:], wgs_r[dt])
                        wu = wgt.tile([128, MS], F32R, tag="wus")
                        nc.sync.dma_start(wu[:], wus_r[dt])
                        for i, (c0, c1) in enumerate(SCH):
                            nc.tensor.matmul(gp[i][:], uTr[:, dt, tt * 128:(tt + 1) * 128],
                                             wg[:, c0:c1], start=(dt == 0), stop=(dt == DT - 1))
                            nc.tensor.matmul(up[i][:], uTr[:, dt, tt * 128:(tt + 1) * 128],
                                             wu[:, c0:c1], start=(dt == 0), stop=(dt == DT - 1))
                    hs_nat = wp.tile([128, MS], F32R, tag="hs_nat", bufs=1)
                    for i, (c0, c1) in enumerate(SCH):
                        sg = wp.tile([128, 512], F32, tag="sg")
                        nc.scalar.activation(sg[:, 0:c1 - c0], gp[i][:], AF.Silu)
                        nc.vector.tensor_tensor(hs_nat[:, c0:c1], sg[:, 0:c1 - c0], up[i][:],
                                                op=OP.mult)
                    for mt in range(MT):
                        tp = ps.tile([128, 128], F32R, tag="shtp", bufs=2)
                        nc.tensor.transpose(tp[:], hs_nat[:, mt * 128:(mt + 1) * 128], ident_r[:])
                        nc.vector.tensor_copy(hsT[:, mt, tt * 128:(tt + 1) * 128], tp[:])

            def shared_down():
                with (
                    tc.tile_pool(name="shB", bufs=2) as wp,
                    tc.tile_pool(name="shBw", bufs=3) as wgt,
                    tc.tile_pool(name="shBps", bufs=1, space="PSUM") as ps,
                ):
                    wds_r = w_ds.rearrange("(mt p) m -> mt p m", p=128)
                    yp = [[ps.tile([128, 512], F32, tag=f"syp{tt}{ch}", name=f"syp{tt}{ch}")
                           for ch in range(4)] for tt in range(2)]
                    for mt in range(MT):
                        wd = wgt.tile([128, D], F32R, tag="wds")
                        nc.sync.dma_start(wd[:], wds_r[mt])
                        for tt in range(2):
                            for ch in range(4):
                                nc.tensor.matmul(yp[tt][ch][:],
                                                 hsT[:, mt, tt * 128:(tt + 1) * 128],
                                                 wd[:, ch * 512:(ch + 1) * 512],
                                                 start=(mt == 0), stop=(mt == MT - 1))
                    for tt in range(2):
                        for ch in range(4):
                            nc.vector.tensor_copy(shared_nat[:, tt, ch * 512:(ch + 1) * 512],
                                                  yp[tt][ch][:])

            shared_gate_up(0)


            # ---------- PHASE 5: routing (redundant on all cores) ----------
            with (
                tc.tile_pool(name="rte", bufs=2) as wp,
                tc.tile_pool(name="rteps", bufs=1, space="PSUM") as ps,
            ):
                fi_sb = wp.tile([128, 16, 4], F32, tag="fi_sb", bufs=1)
                lg = wp.tile([128, 16, E], F32, tag="lgall")
                nc.sync.dma_start(lg[:], lg_ag.rearrange("(p g) e -> p g e", p=128))
                mx = wp.tile([128, 16], F32, tag="mx")
                nc.vector.tensor_reduce(mx[:], lg[:], axis=AX.X, op=OP.max)
                sh_ = wp.tile([128, 16, E], F32, tag="shift")
                nc.vector.tensor_tensor(sh_[:], lg[:], mx[:, :, None].to_broadcast((128, 16, E)),
                                        op=OP.subtract)
                ex = wp.tile([128, 16, E], F32, tag="ex")
                nc.scalar.activation(ex[:], sh_[:], AF.Exp)
                sm = wp.tile([128, 16], F32, tag="sm")
                nc.vector.tensor_reduce(sm[:], ex[:], axis=AX.X, op=OP.add)
                rs = wp.tile([128, 16], F32, tag="rs")
                nc.vector.reciprocal(rs[:], sm[:])
                probs = wp.tile([128, 16, E], F32, tag="probs")
                nc.vector.tensor_tensor(probs[:], ex[:], rs[:, :, None].to_broadcast((128, 16, E)),
                                        op=OP.mult)
                zt = wp.tile([128, 16], F32, tag="zt")
                nc.scalar.activation(zt[:], sm[:], AF.Ln)
                nc.vector.tensor_tensor(zt[:], zt[:], mx[:], op=OP.add)
                nc.vector.tensor_tensor(zt[:], zt[:], zt[:], op=OP.mult)
                z2r = wp.tile([128, 1], F32, tag="z2r")
                nc.vector.tensor_reduce(z2r[:], zt[:], axis=AX.X, op=OP.add)
                z2p = ps.tile([1, 1], F32, tag="z2p")
                nc.tensor.matmul(z2p[:], ones_col[:], z2r[:], start=True, stop=True)
                ind1 = wp.tile([128, 16, E], F32, tag="ind1")
                nc.vector.tensor_tensor(ind1[:], lg[:], mx[:, :, None].to_broadcast((128, 16, E)),
                                        op=OP.is_ge)
                mp1 = wp.tile([128, 16], F32, tag="mp1")
                nc.vector.tensor_reduce(mp1[:], probs[:], axis=AX.X, op=OP.max)
                p2 = wp.tile([128, 16, E], F32, tag="p2")
                nc.vector.tensor_tensor(p2[:], ind1[:], probs[:], op=OP.mult)
                nc.vector.tensor_tensor(p2[:], probs[:], p2[:], op=OP.subtract)
                mp2 = wp.tile([128, 16], F32, tag="mp2")
                nc.vector.tensor_reduce(mp2[:], p2[:], axis=AX.X, op=OP.max)
                ind2 = wp.tile([128, 16, E], F32, tag="ind2")
                nc.vector.tensor_tensor(ind2[:], p2[:], mp2[:, :, None].to_broadcast((128, 16, E)),
                                        op=OP.is_ge)
                ind = wp.tile([128, 16, E], F32, tag="ind")
                nc.vector.tensor_tensor(ind[:], ind1[:], ind2[:], op=OP.add)
                wsum = wp.tile([128, 16], F32, tag="wsum")
                nc.vector.tensor_tensor(wsum[:], mp1[:], mp2[:], op=OP.add)
                nc.vector.reciprocal(wsum[:], wsum[:])
                w1 = wp.tile([128, 16], F32, tag="w1")
                w2 = wp.tile([128, 16], F32, tag="w2")
                nc.vector.tensor_tensor(w1[:], mp1[:], wsum[:], op=OP.mult)
                nc.vector.tensor_tensor(w2[:], mp2[:], wsum[:], op=OP.mult)
                totals = wp.tile([128, E], F32, tag="totals")
                nc.vector.tensor_reduce(totals[:], ind[:].rearrange("p g e -> p e g"),
                                        axis=AX.X, op=OP.add)
                probsum = wp.tile([128, E], F32, tag="probsum")
                nc.vector.tensor_reduce(probsum[:], probs[:].rearrange("p g e -> p e g"),
                                        axis=AX.X, op=OP.add)
                c0 = wp.tile([128, 16, E], F32, tag="c0")
                c1 = wp.tile([128, 16, E], F32, tag="c1")
                nc.vector.tensor_copy(c0[:], ind[:])
                src, dstc = c0, c1
                for shf in (1, 2, 4, 8):
                    nc.vector.tensor_copy(dstc[:, :shf, :], src[:, :shf, :])
                    nc.vector.tensor_tensor(dstc[:, shf:, :], src[:, shf:, :],
                                            src[:, :16 - shf, :], op=OP.add)
                    src, dstc = dstc, src
                incl = src
                excl = wp.tile([128, 16, E], F32, tag="excl")
                nc.vector.tensor_tensor(excl[:], incl[:], ind[:], op=OP.subtract)
                offp = ps.tile([128, E], F32, tag="offp")
                nc.tensor.matmul(offp[:], triu_sb[:], totals[:], start=True, stop=True)
                offs = wp.tile([128, E], F32, tag="offs")
                nc.vector.tensor_copy(offs[:], offp[:])
                pos = wp.tile([128, 16, E], F32, tag="pos")
                nc.vector.tensor_tensor(pos[:], excl[:], offs[:, None, :].to_broadcast((128, 16, E)),
                                        op=OP.add)
                keep = wp.tile([128, 16, E], F32, tag="keep")
                nc.vector.tensor_scalar(keep[:], pos[:], float(CAP), None, op0=OP.is_lt)
                indk = wp.tile([128, 16, E], F32, tag="indk")
                nc.vector.tensor_tensor(indk[:], ind[:], keep[:], op=OP.mult)
                mpos = wp.tile([128, 16, E], F32, tag="mpos")
                nc.vector.tensor_tensor(mpos[:], pos[:], indk[:], op=OP.mult)
                t9 = wp.tile([128, 16, E], F32, tag="t9")
                nc.vector.tensor_scalar(t9[:], indk[:], -9999.0, 9999.0, op0=OP.mult, op1=OP.add)
                nc.vector.tensor_tensor(mpos[:], mpos[:], t9[:], op=OP.add)
                me = wp.tile([128, 16, E], F32, tag="me")
                nc.vector.tensor_tensor(me[:], mpos[:],
                                        onehot_sb[:, None, :].to_broadcast((128, 16, E)), op=OP.mult)
                msl = wp.tile([128, 16], F32, tag="msl")
                nc.vector.tensor_reduce(msl[:], me[:], axis=AX.X, op=OP.add)
                ip1 = ps.tile([1, 512], F32, tag="ip1")
                ip2 = ps.tile([1, CAP - 512], F32, tag="ip2")
                for g in range(16):
                    tval = wp.tile([128, 1], F32, tag="tval")
                    nc.vector.tensor_scalar(tval[:], p16_sb[:], float(g), None, op0=OP.add)
                    eq = wp.tile([128, CAP], F32, tag="eq")
                    nc.vector.tensor_scalar(eq[:], slot_iota_sb[:], msl[:, g:g + 1], None,
                                            op0=OP.is_equal)
                    nc.tensor.matmul(ip1[:], tval[:], eq[:, 0:512], start=(g == 0), stop=(g == 15))
                    nc.tensor.matmul(ip2[:], tval[:], eq[:, 512:CAP], start=(g == 0), stop=(g == 15))
                idx_f = wp.tile([1, CAP], F32, tag="idx_f")
                nc.vector.tensor_copy(idx_f[:, 0:512], ip1[:])
                nc.vector.tensor_copy(idx_f[:, 512:CAP], ip2[:])
                idx_i = wp.tile([1, CAP], I32, tag="idx_i")
                nc.vector.tensor_copy(idx_i[:], idx_f[:])
                nc.sync.dma_start(idx_dram[None, :], idx_i[:])

                def build_fi(indx, wx, col_f, col_w):
                    ec = wp.tile([128, 16, E], F32, tag="ec")
                    nc.vector.tensor_tensor(ec[:], indx[:],
                                            eiota_sb[:, None, :].to_broadcast((128, 16, E)), op=OP.mult)
                    ev = wp.tile([128, 16], F32, tag="ev")
                    nc.vector.tensor_reduce(ev[:], ec[:], axis=AX.X, op=OP.add)
                    sc = wp.tile([128, 16, E], F32, tag="sc")
                    nc.vector.tensor_tensor(sc[:], indx[:], pos[:], op=OP.mult)
                    sv = wp.tile([128, 16], F32, tag="sv")
                    nc.vector.tensor_reduce(sv[:], sc[:], axis=AX.X, op=OP.add)
                    kc = wp.tile([128, 16, E], F32, tag="kc")
                    nc.vector.tensor_tensor(kc[:], indx[:], keep[:], op=OP.mult)
                    kv_ = wp.tile([128, 16], F32, tag="kv_")
                    nc.vector.tensor_reduce(kv_[:], kc[:], axis=AX.X, op=OP.add)
                    fl = wp.tile([128, 16], F32, tag="fl")
                    nc.vector.tensor_scalar(fl[:], ev[:], float(CAP), None, op0=OP.mult)
                    nc.vector.tensor_tensor(fl[:], fl[:], sv[:], op=OP.add)
                    nc.vector.tensor_scalar(fl[:], fl[:], float(E * CAP - 1), None, op0=OP.min)
                    nc.vector.tensor_copy(fi_sb[:, :, col_f], fl[:])
                    wv_t = wp.tile([128, 16], F32, tag="wv_t")
                    nc.vector.tensor_tensor(wv_t[:], wx[:], kv_[:], op=OP.mult)
                    nc.vector.tensor_copy(fi_sb[:, :, col_w], wv_t[:])

                build_fi(ind1, w1, 0, 1)
                build_fi(ind2, w2, 2, 3)
                nc.sync.dma_start(fi_dram.rearrange("(p g) c -> p (g c)", p=128),
                                  fi_sb[:].rearrange("p g c -> p (g c)"))
                cnt_p = ps.tile([1, E], F32, tag="cntp")
                nc.tensor.matmul(cnt_p[:], ones_col[:], totals[:], start=True, stop=True)
                ps_p = ps.tile([1, E], F32, tag="psp")
                nc.tensor.matmul(ps_p[:], ones_col[:], probsum[:], start=True, stop=True)
                st = wp.tile([1, 32], F32, tag="stats")
                nc.vector.memset(st[:], 0.0)
                nc.vector.tensor_copy(st[:, 0:E], cnt_p[:])
                nc.vector.tensor_copy(st[:, 8:8 + E], ps_p[:])
                nc.vector.tensor_copy(st[:, 16:17], z2p[:])
                nc.sync.dma_start(stats_out[:], st[:])

            # ---------- PHASE 6: expert FFN ----------
            with (
                tc.tile_pool(name="ex", bufs=2) as wp,
                tc.tile_pool(name="exps", bufs=2, space="PSUM") as ps,
                tc.tile_pool(name="exps_t", bufs=2, space="PSUM") as ps_t,
                tc.tile_pool(name="exps_y", bufs=1, space="PSUM") as ps_y,
            ):
                idx2 = wp.tile([128, 5], I32, tag="idx2")
                nc.sync.dma_start(idx2[:], idx_dram.rearrange("(g p) -> p g", p=128))
                exA = tc.alloc_tile_pool(name="exA", bufs=2)
                xbT = exA.tile([128, DT, CAP], F32R, tag="xbT", bufs=1)
                for gi in range(5):
                    xb_nat = exA.tile([128, D], mybir.dt.bfloat16, tag="xb_nat")
                    nc.gpsimd.indirect_dma_start(
                        out=xb_nat[:], out_offset=None, in_=u_ag[:],
                        in_offset=IndirectOffsetOnAxis(ap=idx2[:, gi:gi + 1], axis=0))
                    for dt in range(DT):
                        tp = ps_t.tile([128, 128], mybir.dt.bfloat16, tag="extp", bufs=1)
                        nc.tensor.transpose(tp[:], xb_nat[:, dt * 128:(dt + 1) * 128], ident_b[:])
                        nc.vector.tensor_scalar(xbT[:, dt, gi * 128:(gi + 1) * 128], tp[:],
                                                ln2_col[:, dt:dt + 1], None, op0=OP.mult)
                heT = wp.tile([128, MT, CAP], F32R, tag="heT", bufs=1)
                for mt in range(MT):
                    wg = exA.tile([128, DT, 128], F32R, tag="wge", bufs=2)
                    nc.sync.dma_start(wg[:], wge_h[mt])
                    wu = exA.tile([128, DT, 128], F32R, tag="wue", bufs=2)
                    nc.sync.dma_start(wu[:], wue_h[mt])
                    for ch in range(2):
                        cs = slice(ch * 320, (ch + 1) * 320)
                        gp = ps.tile([128, 320], F32, tag="gep")
                        up = ps.tile([128, 320], F32, tag="uep")
                        for dt in range(DT):
                            nc.tensor.matmul(gp[:], wg[:, dt, :], xbT[:, dt, cs],
                                             start=(dt == 0), stop=(dt == DT - 1))
                            nc.tensor.matmul(up[:], wu[:, dt, :], xbT[:, dt, cs],
                                             start=(dt == 0), stop=(dt == DT - 1))
                        sg = wp.tile([128, 320], F32, tag="sge")
                        nc.scalar.activation(sg[:], gp[:], AF.Silu)
                        nc.vector.tensor_tensor(heT[:, mt, cs], sg[:], up[:], op=OP.mult)
                exA.release()
                exB = tc.alloc_tile_pool(name="exB", bufs=2)
                yb_nat = exB.tile([128, 5, D], mybir.dt.bfloat16, tag="yb_nat", bufs=1)
                for dt in range(DT):
                    wd = exB.tile([128, MT, 128], F32R, tag="wde", bufs=2)
                    nc.sync.dma_start(wd[:], wde_h[dt])
                    ypA = ps_y.tile([128, 320], F32, tag="ydpA")
                    ypB = ps_y.tile([128, 320], F32, tag="ydpB")
                    for mt in range(MT):
                        nc.tensor.matmul(ypA[:], wd[:, mt, :], heT[:, mt, 0:320],
                                         start=(mt == 0), stop=(mt == MT - 1))
                        nc.tensor.matmul(ypB[:], wd[:, mt, :], heT[:, mt, 320:CAP],
                                         start=(mt == 0), stop=(mt == MT - 1))
                    ys = exB.tile([128, CAP], F32R, tag="ys")
                    nc.vector.tensor_copy(ys[:, 0:320], ypA[:])
                    nc.vector.tensor_copy(ys[:, 320:CAP], ypB[:])
                    for gi in range(5):
                        tp = ps_t.tile([128, 128], F32R, tag="extpr", bufs=1)
                        nc.tensor.transpose(tp[:], ys[:, gi * 128:(gi + 1) * 128], ident_r[:])
                        nc.vector.tensor_copy(yb_nat[:, gi, dt * 128:(dt + 1) * 128], tp[:])
                for gi in range(5):
                    nc.sync.dma_start(yb_loc[gi * 128:(gi + 1) * 128, :], yb_nat[:, gi, :])
                nc.gpsimd.collective_compute("AllGather", OP.bypass, replica_groups=RG,
                                             ins=[yb_loc[:]], outs=[yb_ag[:]])
                exB.release()

            # (shared part B emitted after expert AG)
            shared_gate_up(1)
            shared_down()

            # ---------- PHASE 7: combine ----------
            with tc.tile_pool(name="cb", bufs=2) as wp:
                fi_my = wp.tile([128, 2, 4], F32, tag="fi_my")
                for tt in range(2):
                    nc.gpsimd.indirect_dma_start(
                        out=fi_my[:, tt, :], out_offset=None, in_=fi_dram[:],
                        in_offset=IndirectOffsetOnAxis(ap=own_rows_sb[:, tt:tt + 1], axis=0))
                of1 = wp.tile([128, 2], I32, tag="of1")
                of2 = wp.tile([128, 2], I32, tag="of2")
                nc.vector.tensor_copy(of1[:], fi_my[:, :, 0])
                nc.vector.tensor_copy(of2[:], fi_my[:, :, 2])
                for tt in range(2):
                    g1 = wp.tile([128, D], mybir.dt.bfloat16, tag="g1")
                    g2 = wp.tile([128, D], mybir.dt.bfloat16, tag="g2")
                    nc.gpsimd.indirect_dma_start(
                        out=g1[:], out_offset=None, in_=yb_ag[:],
                        in_offset=IndirectOffsetOnAxis(ap=of1[:, tt:tt + 1], axis=0))
                    nc.gpsimd.indirect_dma_start(
                        out=g2[:], out_offset=None, in_=yb_ag[:],
                        in_offset=IndirectOffsetOnAxis(ap=of2[:, tt:tt + 1], axis=0))
                    moe = wp.tile([128, D], F32, tag="moe")
                    moe2 = wp.tile([128, D], F32, tag="moe2")
                    nc.vector.tensor_scalar(moe[:], g1[:], fi_my[:, tt, 1:2], None, op0=OP.mult)
                    nc.vector.tensor_scalar(moe2[:], g2[:], fi_my[:, tt, 3:4], None, op0=OP.mult)
                    nc.vector.tensor_tensor(moe[:], moe[:], moe2[:], op=OP.add)
                    yt = wp.tile([128, D], F32, tag="yt")
                    nc.vector.tensor_tensor(yt[:], h_sb[:, tt, :], shared_nat[:, tt, :], op=OP.add)
                    nc.vector.tensor_tensor(yt[:], yt[:], moe[:], op=OP.add)
                    nc.sync.dma_start(y_out[tt * 128:(tt + 1) * 128, :], yt[:])
            shp.release()
            uTp.release()
            uTrp.release()

    nc.compile()
    return nc


_NC_CACHE = None


def _get_program():
    global _NC_CACHE
    if _NC_CACHE is None:
        _NC_CACHE = build_program()
    return _NC_CACHE


def _host_tables():
    inv_freq = 1.0 / THETA ** (np.arange(HALF, dtype=np.float32) / HALF)
    ang = np.arange(S, dtype=np.float32)[:, None] * inv_freq[None, :]
    cos_full = np.concatenate([np.cos(ang).T, np.cos(ang).T]).astype(np.float32)   # [64, S]
    ssin_full = np.concatenate([-np.sin(ang).T, np.sin(ang).T]).astype(np.float32)
    perm_to_true = np.empty(T, dtype=np.int64)
    for s in range(NB):
        tb = true_block(s)
        perm_to_true[s * 128:(s + 1) * 128] = np.arange(tb * 128, tb * 128 + 128)
    triu = np.triu(np.ones((128, 128), np.float32), 1)
    slot_iota = np.broadcast_to(np.arange(CAP, dtype=np.float32), (128, CAP)).copy()
    p16 = (np.arange(128, dtype=np.float32) * 16).reshape(128, 1)
    e_iota = np.broadcast_to(np.arange(E, dtype=np.float32), (128, E)).copy()
    kpos_cols = np.empty((128, NB), np.float32)
    for s in range(NB):
        kpos_cols[:, s] = perm_to_true[s * 128:(s + 1) * 128]
    return cos_full, ssin_full, perm_to_true, triu, slot_iota, p16, e_iota, kpos_cols


def _col_major(w, n_outer, q=128):
    """[D_in, n_outer*q] -> [n_outer, 128, D_in//128, q] with
    (o, p, t, c) = w[t*128+p, o*q+c]; contiguous per (o, p)."""
    d_in = w.shape[0]
    return np.ascontiguousarray(
        w.reshape(d_in // 128, 128, n_outer, q).transpose(2, 1, 0, 3))


def kernel(x, ln1_w, ln2_w, w_q, w_k, w_v, w_o, attn_gate, w_router,
           w_gate_e, w_up_e, w_down_e, w_gate_s, w_up_s, w_down_s,
           _trace=False):
    nc = _get_program()
    cos_full, ssin_full, perm_to_true, triu, slot_iota, p16, e_iota, kpos_cols = _host_tables()

    f32 = lambda a: np.ascontiguousarray(np.asarray(a, dtype=np.float32))
    x2 = f32(x).reshape(T, D)
    w_gate_e, w_up_e, w_down_e = f32(w_gate_e), f32(w_up_e), f32(w_down_e)
    shared_inputs = dict(
        x_nat=None,
        w_q=f32(w_q), w_k=f32(w_k),
        w_v=f32(w_v), w_o=f32(w_o),
        attn_gate=f32(attn_gate),
        wr_h=np.ascontiguousarray(f32(w_router).reshape(DT, 128, E).transpose(1, 0, 2)),
        w_gs=f32(w_gate_s), w_us=f32(w_up_s), w_ds=f32(w_down_s),
        ln1_w=f32(ln1_w), ln2_w=f32(ln2_w),
        triu=triu, slot_iota=slot_iota, p16_col=p16, e_iota=e_iota,
        kpos_cols=kpos_cols,
    )
    in_maps = []
    for c in range(NC):
        rows = perm_to_true[c * TPC:(c + 1) * TPC]
        m = dict(shared_inputs)
        m["x_nat"] = np.ascontiguousarray(x2[rows])
        m["wge_h"] = _col_major(w_gate_e[c], MT)
        m["wue_h"] = _col_major(w_up_e[c], MT)
        m["wde_h"] = _col_major(w_down_e[c], DT)
        m["cos2_t"] = np.ascontiguousarray(cos_full[:, rows])
        m["ssin2_t"] = np.ascontiguousarray(ssin_full[:, rows])
        m["qpos_bc"] = np.ascontiguousarray(
            np.broadcast_to(rows.astype(np.float32), (128, TPC)))
        m["onehot_e"] = np.broadcast_to(
            (np.arange(E) == c).astype(np.float32), (128, E)).copy()
        m["own_rows"] = np.ascontiguousarray(
            (c * TPC + np.arange(TPC, dtype=np.int32)).reshape(2, 128).T)
        in_maps.append(m)

    res = run_bass_kernel_spmd(nc, in_maps, core_ids=list(range(NC)), trace=_trace)

    y = np.empty((T, D), np.float32)
    for c in range(NC):
        rows = perm_to_true[c * TPC:(c + 1) * TPC]
        y[rows] = res.results[c]["y_out"]
    st = res.results[0]["stats_out"][0]
    counts, probsum, z2 = st[0:E], st[8:8 + E], st[16]
    frac = counts / max(float(counts.sum()), 1.0)
    mean_probs = probsum / T
    lb_loss = E * float((frac * K * mean_probs).sum())
    z_loss = float(z2) / T
    aux = np.float32(0.01 * lb_loss + 0.001 * z_loss)
    out = (y.reshape(B, S, D), aux)
    if _trace:
        return out, res
    return out


# revision 27
# speedup vs baseline: 1.1065x; 1.1065x over previous
"""Trainium2 Bass kernel for nn_Block_78280073937290 (moe_routing).

8-core SPMD plan:
- Token-parallel attention; core c owns true 128-token blocks {c, 15-c}
  (causal-balanced).  All per-core variation is input DATA (the program is
  identical on every core).
- fp32r (tf32-precision, full-rate) matmuls on the attention/shared/expert
  paths; true fp32 matmuls for router logits (top-2 selection is
  precision-critical) and integer-valued index builds.
- MoE: router logits AllGathered; routing/slots computed redundantly on all
  cores from identical fp32 logits; each core indirect-gathers the tokens
  routed to its expert from the AllGathered u, runs the expert FFN, AllGathers
  the compact outputs, and combines its own tokens' two expert rows locally.
- aux-loss partials are computed on device; the scalar is assembled on host.
"""

import numpy as np

import concourse.bass as bass
import concourse.mybir as mybir
import concourse.tile as tile
from concourse import bacc
from concourse.bass import IndirectOffsetOnAxis
from concourse.bass_utils import run_bass_kernel_spmd
from concourse.masks import make_identity

F32 = mybir.dt.float32
F32R = mybir.dt.float32r
I32 = mybir.dt.int32
AF = mybir.ActivationFunctionType
OP = mybir.AluOpType
AX = mybir.AxisListType

B, S, D = 1, 2048, 2048
N, MKV, HD = 16, 4, 128
E, K = 8, 2
MI, MS = 1408, 1408
G = 12
ROT = HD // 2          # 64
HALF = ROT // 2        # 32
THETA = 1024.0
CAP = int(B * S * K / E * 1.25)  # 640
EPS = 1e-5
T = B * S              # 2048
NC = 8
TPC = T // NC          # 256
NB = S // 128          # 16
DT = D // 128          # 16
MT = MI // 128         # 11
INV_SQRT_HD = 1.0 / float(np.sqrt(HD))


def true_block(s):
    return s // 2 if s % 2 == 0 else 15 - s // 2


def r32(ap):
    return ap.bitcast(F32R)


def build_program():
    nc = bacc.Bacc("TRN2", target_bir_lowering=False, debug=False, num_devices=NC)

    def inp(name, shape, dtype=F32):
        return nc.dram_tensor(name, shape, dtype, kind="ExternalInput").ap()

    def inp_r(name, shape):
        return nc.dram_tensor(name, shape, F32R, kind="ExternalInput").ap()

    # activations / tables (per-core data)
    x_nat = inp("x_nat", [TPC, D])
    cos2_t = inp("cos2_t", [ROT, TPC])
    ssin2_t = inp("ssin2_t", [ROT, TPC])
    qpos_bc = inp("qpos_bc", [128, TPC])
    kpos_cols = inp("kpos_cols", [128, NB])
    triu = inp("triu", [128, 128])
    swap64_t = inp_r("swap64_t", [ROT, ROT])
    slot_iota = inp("slot_iota", [128, CAP])
    p16_col = inp("p16_col", [128, 1])
    onehot_e = inp("onehot_e", [128, E])
    e_iota = inp("e_iota", [128, E])
    own_rows = inp("own_rows", [128, 2], I32)
    ln1_w = inp("ln1_w", [D])
    ln2_w = inp("ln2_w", [D])
    attn_gate = inp_r("attn_gate", [G, N])
    # weights, host pre-laid-out for contiguous per-partition DMA
    w_q = inp_r("w_q", [D, N * HD])              # natural
    w_k = inp_r("w_k", [D, MKV * HD])
    w_v = inp_r("w_v", [D, MKV * HD])
    w_o = inp_r("w_o", [N * HD, D])
    w_router = inp("wr_h", [128, DT, E])       # (p,dt,e) = w_router[dt*128+p, e]
    w_gs = inp_r("w_gs", [D, MS])
    w_us = inp_r("w_us", [D, MS])
    w_ds = inp_r("w_ds", [MS, D])
    wge_h = inp_r("wge_h", [MT, 128, DT, 128])
    wue_h = inp_r("wue_h", [MT, 128, DT, 128])
    wde_h = inp_r("wde_h", [DT, 128, MT, 128])

    y_out = nc.dram_tensor("y_out", [TPC, D], F32, kind="ExternalOutput").ap()
    stats_out = nc.dram_tensor("stats_out", [1, 32], F32, kind="ExternalOutput").ap()
    dbg_logits = nc.dram_tensor("dbg_logits", [T, E], F32, kind="ExternalOutput").ap()

    kT_loc = nc.dram_tensor("kT_loc", [MKV, 128, TPC], F32R).ap()
    kT_ag = nc.dram_tensor("kT_ag", [NC * MKV, 128, TPC], F32R, addr_space="Shared").ap()
    # v stored partition-major per kv head: (kvh, p, tt, hd)
    v_loc = nc.dram_tensor("v_loc", [MKV, 128, 2, HD], F32R).ap()
    v_ag = nc.dram_tensor("v_ag", [NC * MKV, 128, 2, HD], F32R, addr_space="Shared").ap()
    u_loc = nc.dram_tensor("u_loc", [TPC, D], mybir.dt.bfloat16).ap()
    u_ag = nc.dram_tensor("u_ag", [T, D], mybir.dt.bfloat16, addr_space="Shared").ap()
    lg_loc = nc.dram_tensor("lg_loc", [TPC, E], F32).ap()
    lg_ag = nc.dram_tensor("lg_ag", [T, E], F32, addr_space="Shared").ap()
    yb_loc = nc.dram_tensor("yb_loc", [CAP, D], mybir.dt.bfloat16).ap()
    yb_ag = nc.dram_tensor("yb_ag", [E * CAP, D], mybir.dt.bfloat16, addr_space="Shared").ap()
    idx_dram = nc.dram_tensor("idx_dram", [CAP], I32).ap()
    fi_dram = nc.dram_tensor("fi_dram", [T, 4], F32).ap()

    RG = [list(range(NC))]

    with tile.TileContext(nc) as tc:
        with (
            tc.tile_pool(name="persist", bufs=1) as pp,
            tc.tile_pool(name="const", bufs=1) as cp,
        ):
            ident = cp.tile([128, 128], F32, tag="ident")
            make_identity(nc, ident)
            ident_r = cp.tile([128, 128], F32R, tag="ident_r")
            ones_col_r = cp.tile([128, 1], F32R, tag="ones_col_r")
            ones_col = cp.tile([128, 1], F32, tag="ones_col")
            nc.vector.memset(ones_col[:], 1.0)
            ones_row = cp.tile([1, 128], F32, tag="ones_row")
            nc.vector.memset(ones_row[:], 1.0)
            nc.vector.tensor_copy(ident_r[:], ident[:])
            ident_b = cp.tile([128, 128], mybir.dt.bfloat16, tag="ident_b")
            nc.vector.tensor_copy(ident_b[:], ident[:])
            swap64 = cp.tile([ROT, ROT], F32R, tag="swap64")
            nc.sync.dma_start(swap64[:], swap64_t)
            nc.vector.tensor_copy(ones_col_r[:], ones_col[:])
            eps_ln = cp.tile([128, 1], F32, tag="eps_ln")
            nc.vector.memset(eps_ln[:], EPS)
            eps_qk = cp.tile([1, 1], F32, tag="eps_qk")
            nc.vector.memset(eps_qk[:], 1e-6)

            def load_const(ap_dram, shape, tag, dtype=F32):
                t = cp.tile(shape, dtype, tag=tag)
                nc.sync.dma_start(t[:], ap_dram)
                return t

            cos2_sb = load_const(cos2_t, [ROT, TPC], "cos2")
            ssin2_sb = load_const(ssin2_t, [ROT, TPC], "ssin2")
            qpos_sb = load_const(qpos_bc, [128, TPC], "qpos")
            kpos_sb = load_const(kpos_cols, [128, NB], "kpos")
            triu_sb = load_const(triu, [128, 128], "triu")
            slot_iota_sb = load_const(slot_iota, [128, CAP], "slot_iota")
            p16_sb = load_const(p16_col, [128, 1], "p16")
            onehot_sb = load_const(onehot_e, [128, E], "onehot")
            eiota_sb = load_const(e_iota, [128, E], "eiota")
            own_rows_sb = load_const(own_rows, [128, 2], "own_rows", I32)
            ln1_col = load_const(ln1_w.rearrange("(o p) -> p o", p=128), [128, DT], "ln1col")
            ln2_col = load_const(ln2_w.rearrange("(o p) -> p o", p=128), [128, DT], "ln2col")
            ag_sb = load_const(attn_gate, [G, N], "ag", F32R)

            h_sb = pp.tile([128, 2, D], F32, tag="h_sb")

            uTrp = tc.alloc_tile_pool(name="uTrp", bufs=1)
            uTr = uTrp.tile([128, DT, TPC], F32R, tag="uTr")
            uTp = tc.alloc_tile_pool(name="uTp", bufs=1)
            uT = uTp.tile([128, DT, TPC], F32, tag="uT")
            actp = tc.alloc_tile_pool(name="actp", bufs=1)
            u1T = actp.tile([128, DT, TPC], F32R, tag="u1T")
            qT = actp.tile([128, N, TPC], F32R, tag="qT")
            oT = actp.tile([128, N, TPC], F32R, tag="oT")
            gZ = actp.tile([1, N, TPC], F32, tag="gZ")

            # ---------- PHASE 0: u1 = rmsnorm(x); u1T ----------
            with (
                tc.tile_pool(name="ph0", bufs=2) as wp,
                tc.tile_pool(name="ph0ps", bufs=2, space="PSUM") as ps,
            ):
                u1_nat = wp.tile([128, 2, D], F32, tag="u1nat", bufs=1)
                x_ph0 = wp.tile([128, 2, D], F32, tag="x_ph0", bufs=1)
                for tt in range(2):
                    nc.sync.dma_start(x_ph0[:, tt, :], x_nat[tt * 128:(tt + 1) * 128, :])
                for tt in range(2):
                    sq = wp.tile([128, D], F32, tag="sq")
                    nc.vector.tensor_tensor(sq[:], x_ph0[:, tt, :], x_ph0[:, tt, :], op=OP.mult)
                    ssum = wp.tile([128, 1], F32, tag="ssum")
                    nc.vector.tensor_reduce(ssum[:], sq[:], axis=AX.X, op=OP.add)
                    rt = wp.tile([128, 1], F32, tag="rt")
                    nc.scalar.activation(rt[:], ssum[:], AF.Sqrt, scale=1.0 / D, bias=eps_ln[:])
                    nc.vector.reciprocal(rt[:], rt[:])
                    nc.vector.tensor_scalar(u1_nat[:, tt, :], x_ph0[:, tt, :], rt[:], None, op0=OP.mult)
                for dt in range(DT):
                    for tt in range(2):
                        pt = ps.tile([128, 128], F32, tag="tp")
                        nc.tensor.transpose(pt[:], u1_nat[:, tt, dt * 128:(dt + 1) * 128], ident[:])
                        nc.vector.tensor_scalar(u1T[:, dt, tt * 128:(tt + 1) * 128], pt[:],
                                                ln1_col[:, dt:dt + 1], None, op0=OP.mult)

            # ---------- PHASE 1: QKV (natural, N=512 moving), norms, rope ----------
            with (
                tc.tile_pool(name="ph1", bufs=2) as wp,
                tc.tile_pool(name="ph1r", bufs=1) as rp,
                tc.tile_pool(name="ph1w", bufs=2) as wgt,
            ):
                rq_flat = rp.tile([1, N, TPC], F32, tag="rq_flat")
                rk_flat = rp.tile([1, MKV, TPC], F32, tag="rk_flat")
                wk_r = w_k.rearrange("(dt p) m -> dt p m", p=128)
                wv_r = w_v.rearrange("(dt p) m -> dt p m", p=128)
                wq_r = w_q.rearrange("(dt p) m -> dt p m", p=128)

                # --- pass 1: k_nat, v_nat ---
                ps_kv = tc.alloc_tile_pool(name="ps_kv", bufs=1, space="PSUM")
                pk = [ps_kv.tile([128, MKV * HD], F32, tag=f"pk{tt}", name=f"pk{tt}")
                      for tt in range(2)]
                pv = [ps_kv.tile([128, MKV * HD], F32, tag=f"pv{tt}", name=f"pv{tt}")
                      for tt in range(2)]
                for dt in range(DT):
                    wtk = wgt.tile([128, MKV * HD], F32R, tag="wtk")
                    nc.sync.dma_start(wtk[:], wk_r[dt])
                    wtv = wgt.tile([128, MKV * HD], F32R, tag="wtv")
                    nc.sync.dma_start(wtv[:], wv_r[dt])
                    for tt in range(2):
                        nc.tensor.matmul(pk[tt][:], u1T[:, dt, tt * 128:(tt + 1) * 128],
                                         wtk[:], start=(dt == 0), stop=(dt == DT - 1))
                        nc.tensor.matmul(pv[tt][:], u1T[:, dt, tt * 128:(tt + 1) * 128],
                                         wtv[:], start=(dt == 0), stop=(dt == DT - 1))
                k_nat = rp.tile([128, 2, MKV * HD], F32R, tag="k_nat")
                v_sb = rp.tile([128, 2, MKV * HD], F32R, tag="v_sb")
                for tt in range(2):
                    nc.vector.tensor_copy(k_nat[:, tt, :], pk[tt][:])
                    nc.vector.tensor_copy(v_sb[:, tt, :], pv[tt][:])
                ps_kv.release()

                ps_m1 = tc.alloc_tile_pool(name="ps_m1", bufs=1, space="PSUM")
                for kh in range(MKV):
                    for tt in range(2):
                        nc.sync.dma_start(v_loc[kh, :, tt, :], v_sb[:, tt, kh * HD:(kh + 1) * HD])
                nc.gpsimd.collective_compute("AllGather", OP.bypass, replica_groups=RG,
                                             ins=[v_loc[:]], outs=[v_ag[:]])

                def qknorm_rope(dst, rdst, wp=wp, ps1=None):
                    """dst: F32R sbuf [128, TPC] (in place); rdst [1, TPC] @p0."""
                    sq = wp.tile([128, TPC], F32, tag="sqh")
                    nc.vector.tensor_tensor(sq[:], dst, dst, op=OP.mult)
                    sp = ps1.tile([1, TPC], F32, tag="normsum", bufs=1)
                    nc.tensor.matmul(sp[:], ones_col[:], sq[:], start=True, stop=True)
                    nc.scalar.activation(rdst, sp[:], AF.Sqrt, scale=1.0 / HD, bias=eps_qk[:])
                    nc.vector.reciprocal(rdst, rdst)
                    xsp = ps1.tile([ROT, TPC], F32, tag="xsp", bufs=2)
                    nc.tensor.matmul(xsp[:], swap64[:], dst[0:ROT, :], start=True, stop=True)
                    tmp = wp.tile([ROT, TPC], F32, tag="tmp_rope")
                    nc.vector.tensor_tensor(tmp[:], xsp[:], ssin2_sb[:], op=OP.mult)
                    xc = wp.tile([ROT, TPC], F32, tag="xc")
                    nc.vector.tensor_tensor(xc[:], dst[0:ROT, :], cos2_sb[:], op=OP.mult)
                    nc.vector.tensor_tensor(dst[0:ROT, :], xc[:], tmp[:], op=OP.add)

                def fold_scale(dst, row_ap, psx):
                    bp = psx.tile([128, TPC], F32, tag="bcast", bufs=2)
                    nc.tensor.matmul(bp[:], ones_row[:], row_ap, start=True, stop=True)
                    bs = wp.tile([128, TPC], F32, tag="bcast_sb")
                    nc.scalar.copy(bs[:], bp[:])
                    nc.vector.tensor_tensor(dst, dst, bs[:], op=OP.mult)

                kT_sb = rp.tile([128, MKV, TPC], F32R, tag="kT_sb")
                for kh in range(MKV):
                    for tt in range(2):
                        ktp = ps_m1.tile([128, 128], F32R, tag="ktp", bufs=2)
                        nc.tensor.transpose(ktp[:], k_nat[:, tt, kh * 128:(kh + 1) * 128],
                                            ident_r[:])
                        nc.scalar.copy(kT_sb[:, kh, tt * 128:(tt + 1) * 128], ktp[:])
                    qknorm_rope(kT_sb[:, kh, :], rk_flat[:, kh, :], ps1=ps_m1)
                    fold_scale(kT_sb[:, kh, :], rk_flat[:, kh, :], ps_m1)
                    nc.sync.dma_start(kT_loc[kh], kT_sb[:, kh, :])
                nc.gpsimd.collective_compute("AllGather", OP.bypass, replica_groups=RG,
                                             ins=[kT_loc[:]], outs=[kT_ag[:]])
                ps_m1.release()

                # --- pass 2: q_nat ---
                ps_q = tc.alloc_tile_pool(name="ps_q", bufs=1, space="PSUM")
                pq = [[ps_q.tile([128, 512], F32, tag=f"pq{tt}{ch}", name=f"pq{tt}{ch}")
                       for ch in range(4)] for tt in range(2)]
                for dt in range(DT):
                    wtq = wgt.tile([128, N * HD], F32R, tag="wtq")
                    nc.sync.dma_start(wtq[:], wq_r[dt])
                    for tt in range(2):
                        for ch in range(4):
                            nc.tensor.matmul(pq[tt][ch][:],
                                             u1T[:, dt, tt * 128:(tt + 1) * 128],
                                             wtq[:, ch * 512:(ch + 1) * 512],
                                             start=(dt == 0), stop=(dt == DT - 1))
                q_nat = rp.tile([128, 2, N * HD], F32R, tag="q_nat")
                for tt in range(2):
                    for ch in range(4):
                        nc.vector.tensor_copy(q_nat[:, tt, ch * 512:(ch + 1) * 512],
                                              pq[tt][ch][:])
                ps_q.release()

                ps_m2 = tc.alloc_tile_pool(name="ps_m2", bufs=1, space="PSUM")
                for h in range(N):
                    for tt in range(2):
                        qtp = ps_m2.tile([128, 128], F32R, tag="qtp", bufs=2)
                        nc.tensor.transpose(qtp[:], q_nat[:, tt, h * 128:(h + 1) * 128],
                                            ident_r[:])
                        nc.scalar.copy(qT[:, h, tt * 128:(tt + 1) * 128], qtp[:])
                    qknorm_rope(qT[:, h, :], rq_flat[:, h, :], ps1=ps_m2)
                    fold_scale(qT[:, h, :], rq_flat[:, h, :], ps_m2)
                # attention output gate
                for h in range(N):
                    gp = ps_m2.tile([1, TPC], F32, tag="gTp", bufs=1)
                    nc.tensor.matmul(gp[:], ag_sb[:, h:h + 1], u1T[0:G, 0, :],
                                     start=True, stop=True)
                    nc.scalar.activation(gZ[:, h, :], gp[:], AF.Sigmoid)
                nc.vector.tensor_scalar(gZ[:].rearrange("o h t -> o (h t)"),
                                        gZ[:].rearrange("o h t -> o (h t)"),
                                        2.0, None, op0=OP.mult)
                ps_m2.release()

            # ---------- PHASE 2: attention core ----------
            with (
                tc.tile_pool(name="att", bufs=2) as wp,
                tc.tile_pool(name="attkv", bufs=1) as kvp,
                tc.tile_pool(name="attpt", bufs=16) as ptp,
                tc.tile_pool(name="attps_s", bufs=2, space="PSUM") as ps_s,
                tc.tile_pool(name="attps_t", bufs=2, space="PSUM") as ps_t,
                tc.tile_pool(name="attps_o", bufs=2, space="PSUM") as ps_o,
                tc.tile_pool(name="attps_z", bufs=1, space="PSUM") as ps_z,
                tc.tile_pool(name="attps_b", bufs=1, space="PSUM") as ps_b,
            ):
                m01T = kvp.tile([128, NB, TPC], mybir.dt.bfloat16, tag="m01T")
                for s in range(NB):
                    nc.vector.tensor_scalar(m01T[:, s, :], qpos_sb[:], kpos_sb[:, s:s + 1],
                                            None, op0=OP.is_ge)
                for kvh in range(MKV):
                    kT_h = kvp.tile([128, NB * 128], F32R, tag="kT_h")
                    for c in range(NC):
                        nc.sync.dma_start(kT_h[:, c * 256:(c + 1) * 256],
                                          kT_ag[c * MKV + kvh, :, :])
                    v_h = kvp.tile([128, NB, HD], F32R, tag="v_h")
                    for c in range(NC):
                        nc.sync.dma_start(v_h[:, 2 * c:2 * c + 2, :],
                                          v_ag[c * MKV + kvh, :, :, :])
                    kT_h3 = kT_h[:].rearrange("p (s q) -> p s q", q=128)
                    for qh in range(4):
                        h = kvh * 4 + qh
                        probsT = [ptp.tile([128, TPC], F32R, tag="probsT", name=f"probsT{h}_{si}")
                                  for si in range(NB)]
                        for qb in range(2):
                            nslots = 8 if qb == 0 else NB
                            pt_buf = wp.tile([128, NB * 128], F32R, tag="ptbuf", bufs=1)
                            nmm = 2 if qb == 0 else 4
                            for m in range(nmm):
                                sp = ps_s.tile([128, 512], F32, tag="scores")
                                if qb == 0:
                                    rhs = kT_h3[:, ::2, :][:, m * 4:(m + 1) * 4, :]
                                else:
                                    rhs = kT_h[:, m * 512:(m + 1) * 512]
                                nc.tensor.matmul(sp[:], qT[:, h, qb * 128:(qb + 1) * 128],
                                                 rhs, start=True, stop=True)
                                nc.scalar.activation(pt_buf[:, m * 512:(m + 1) * 512], sp[:],
                                                     AF.Exp, scale=INV_SQRT_HD)
                            for i in range(nslots):
                                s = 2 * i if qb == 0 else i
                                tp = ps_t.tile([128, 128], F32R, tag="trp")
                                nc.tensor.transpose(tp[:], pt_buf[:, i * 128:(i + 1) * 128], ident_r[:])
                                nc.vector.tensor_tensor(
                                    probsT[s][:, qb * 128:(qb + 1) * 128], tp[:],
                                    m01T[:, s, qb * 128:(qb + 1) * 128], op=OP.mult)
                        zp = ps_z.tile([1, TPC], F32, tag="z")
                        op_ = ps_o.tile([128, TPC], F32, tag="oTp")
                        for s in range(NB):
                            if s % 2 == 0:
                                rhs, zsl, osl = probsT[s][:], zp[:], op_[:]
                            else:
                                rhs = probsT[s][:, 128:TPC]
                                zsl, osl = zp[:, 128:TPC], op_[:, 128:TPC]
                            first, last = s == 0, s == NB - 1
                            nc.tensor.matmul(zsl, ones_col_r[:], rhs, start=first, stop=last)
                            nc.tensor.matmul(osl, v_h[:, s, :], rhs, start=first, stop=last)
                        srow = wp.tile([1, TPC], F32, tag="srow")
                        nc.vector.reciprocal(srow[:], zp[:])
                        nc.vector.tensor_tensor(srow[:], srow[:], gZ[:, h, :], op=OP.mult)
                        bp = ps_b.tile([128, TPC], F32, tag="sb_bc")
                        nc.tensor.matmul(bp[:], ones_row[:], srow[:], start=True, stop=True)
                        bs = wp.tile([128, TPC], F32, tag="sb_bc_sb")
                        nc.vector.tensor_copy(bs[:], bp[:])
                        nc.vector.tensor_tensor(oT[:, h, :], op_[:], bs[:], op=OP.mult)

            # ---------- PHASE 2b: w_o, residual, ln2, uT ----------
            with (
                tc.tile_pool(name="wo", bufs=2) as wgt,
                tc.tile_pool(name="wops", bufs=1, space="PSUM") as ps,
            ):
                wo_r = w_o.rearrange("(h p) d -> h p d", p=128)
                x_rb = wgt.tile([128, 2, D], F32, tag="x_rb", bufs=1)
                for tt in range(2):
                    nc.sync.dma_start(x_rb[:, tt, :], x_nat[tt * 128:(tt + 1) * 128, :])
                pgrid = [[ps.tile([128, 512], F32, tag=f"wop{qb}{ch}", name=f"wop{qb}{ch}")
                          for ch in range(4)] for qb in range(2)]
                for h in range(N):
                    wo_h = wgt.tile([128, D], F32R, tag="wo_h")
                    nc.sync.dma_start(wo_h[:], wo_r[h])
                    for qb in range(2):
                        for ch in range(4):
                            nc.tensor.matmul(pgrid[qb][ch][:],
                                             oT[:, h, qb * 128:(qb + 1) * 128],
                                             wo_h[:, ch * 512:(ch + 1) * 512],
                                             start=(h == 0), stop=(h == N - 1))
                for qb in range(2):
                    for ch in range(4):
                        nc.vector.tensor_tensor(h_sb[:, qb, ch * 512:(ch + 1) * 512],
                                                pgrid[qb][ch][:],
                                                x_rb[:, qb, ch * 512:(ch + 1) * 512], op=OP.add)
            with (
                tc.tile_pool(name="ln2", bufs=2) as wp,
                tc.tile_pool(name="ln2ps", bufs=2, space="PSUM") as ps,
            ):
                u_sb = wp.tile([128, 2, D], F32, tag="u_sb", bufs=1)
                for tt in range(2):
                    sq = wp.tile([128, D], F32, tag="sq2")
                    nc.vector.tensor_tensor(sq[:], h_sb[:, tt, :], h_sb[:, tt, :], op=OP.mult)
                    ssum = wp.tile([128, 1], F32, tag="ssum2")
                    nc.vector.tensor_reduce(ssum[:], sq[:], axis=AX.X, op=OP.add)
                    rt = wp.tile([128, 1], F32, tag="rt2")
                    nc.scalar.activation(rt[:], ssum[:], AF.Sqrt, scale=1.0 / D, bias=eps_ln[:])
                    nc.vector.reciprocal(rt[:], rt[:])
                    nc.vector.tensor_scalar(u_sb[:, tt, :], h_sb[:, tt, :], rt[:], None, op0=OP.mult)
                    u_bf = wp.tile([128, D], mybir.dt.bfloat16, tag="u_bf")
                    nc.vector.tensor_copy(u_bf[:], u_sb[:, tt, :])
                    nc.sync.dma_start(u_loc[tt * 128:(tt + 1) * 128, :], u_bf[:])
                for dt in range(DT):
                    for tt in range(2):
                        pt = ps.tile([128, 128], F32, tag="tpu")
                        nc.tensor.transpose(pt[:], u_sb[:, tt, dt * 128:(dt + 1) * 128], ident[:])
                        nc.vector.tensor_scalar(uT[:, dt, tt * 128:(tt + 1) * 128], pt[:],
                                                ln2_col[:, dt:dt + 1], None, op0=OP.mult)
                        nc.vector.tensor_scalar(uTr[:, dt, tt * 128:(tt + 1) * 128], pt[:],
                                                ln2_col[:, dt:dt + 1], None, op0=OP.mult)
            actp.release()

            # ---------- PHASE 3: router logits (fp32), AGs ----------
            with (
                tc.tile_pool(name="rt", bufs=2) as wp,
                tc.tile_pool(name="rtps", bufs=2, space="PSUM") as ps,
            ):
                wr_sb = wp.tile([128, DT, E], F32, tag="wr")
                nc.sync.dma_start(wr_sb[:], w_router[:])
                lg_sb = wp.tile([128, 2, E], F32, tag="lg")
                for tt in range(2):
                    lp = ps.tile([128, E], F32, tag="lgp")
                    for dt in range(DT):
                        nc.tensor.matmul(lp[:], uT[:, dt, tt * 128:(tt + 1) * 128],
                                         wr_sb[:, dt, :], start=(dt == 0), stop=(dt == DT - 1))
                    nc.vector.tensor_copy(lg_sb[:, tt, :], lp[:])
                    nc.sync.dma_start(lg_loc[tt * 128:(tt + 1) * 128, :], lg_sb[:, tt, :])
                nc.gpsimd.collective_compute("AllGather", OP.bypass, replica_groups=RG,
                                             ins=[lg_loc[:]], outs=[lg_ag[:]])
                nc.gpsimd.collective_compute("AllGather", OP.bypass, replica_groups=RG,
                                             ins=[u_loc[:]], outs=[u_ag[:]])
                nc.sync.dma_start(dbg_logits[:], lg_ag[:])

            # ---------- shared expert (natural layout, split for AG overlap) ----
            shp = tc.alloc_tile_pool(name="shp", bufs=1)
            hsT = shp.tile([128, MT, TPC], F32R, tag="hsT")
            shared_nat = shp.tile([128, 2, D], F32, tag="shared_nat")
            SCH = [(0, 512), (512, 1024), (1024, MS)]

            def shared_gate_up(tt):
                with (
                    tc.tile_pool(name=f"shA{tt}", bufs=2) as wp,
                    tc.tile_pool(name=f"shAw{tt}", bufs=3) as wgt,
                    tc.tile_pool(name=f"shAps{tt}", bufs=1, space="PSUM") as ps,
                ):
                    wgs_r = w_gs.rearrange("(dt p) m -> dt p m", p=128)
                    wus_r = w_us.rearrange("(dt p) m -> dt p m", p=128)
                    gp = [ps.tile([128, c1 - c0], F32, tag=f"sgp{i}", name=f"sgp{i}")
                          for i, (c0, c1) in enumerate(SCH)]
                    up = [ps.tile([128, c1 - c0], F32, tag=f"sup{i}", name=f"sup{i}")
                          for i, (c0, c1) in enumerate(SCH)]
                    for dt in range(DT):
                        wg = wgt.tile([128, MS], F32R, tag="wgs")
                        nc.sync.dma_start(wg[:], wgs_r[dt])
                        wu = wgt.tile([128, MS], F32R, tag="wus")
                        nc.sync.dma_start(wu[:], wus_r[dt])
                        for i, (c0, c1) in enumerate(SCH):
                            nc.tensor.matmul(gp[i][:], uTr[:, dt, tt * 128:(tt + 1) * 128],
                                             wg[:, c0:c1], start=(dt == 0), stop=(dt == DT - 1))
                            nc.tensor.matmul(up[i][:], uTr[:, dt, tt * 128:(tt + 1) * 128],
                                             wu[:, c0:c1], start=(dt == 0), stop=(dt == DT - 1))
                    hs_nat = wp.tile([128, MS], F32R, tag="hs_nat", bufs=1)
                    for i, (c0, c1) in enumerate(SCH):
                        sg = wp.tile([128, 512], F32, tag="sg")
                        nc.scalar.activation(sg[:, 0:c1 - c0], gp[i][:], AF.Silu)
                        nc.vector.tensor_tensor(hs_nat[:, c0:c1], sg[:, 0:c1 - c0], up[i][:],
                                                op=OP.mult)
                    for mt in range(MT):
                        tp = ps.tile([128, 128], F32R, tag="shtp", bufs=2)
                        nc.tensor.transpose(tp[:], hs_nat[:, mt * 128:(mt + 1) * 128], ident_r[:])
                        nc.vector.tensor_copy(hsT[:, mt, tt * 128:(tt + 1) * 128], tp[:])

            def shared_down():
                with (
                    tc.tile_pool(name="shB", bufs=2) as wp,
                    tc.tile_pool(name="shBw", bufs=3) as wgt,
                    tc.tile_pool(name="shBps", bufs=1, space="PSUM") as ps,
                ):
                    wds_r = w_ds.rearrange("(mt p) m -> mt p m", p=128)
                    yp = [[ps.tile([128, 512], F32, tag=f"syp{tt}{ch}", name=f"syp{tt}{ch}")
                           for ch in range(4)] for tt in range(2)]
                    for mt in range(MT):
                        wd = wgt.tile([128, D], F32R, tag="wds")
                        nc.sync.dma_start(wd[:], wds_r[mt])
                        for tt in range(2):
                            for ch in range(4):
                                nc.tensor.matmul(yp[tt][ch][:],
                                                 hsT[:, mt, tt * 128:(tt + 1) * 128],
                                                 wd[:, ch * 512:(ch + 1) * 512],
                                                 start=(mt == 0), stop=(mt == MT - 1))
                    for tt in range(2):
                        for ch in range(4):
                            nc.vector.tensor_copy(shared_nat[:, tt, ch * 512:(ch + 1) * 512],
                                                  yp[tt][ch][:])

            shared_gate_up(0)


            # ---------- PHASE 5: routing (redundant on all cores) ----------
            with (
                tc.tile_pool(name="rte", bufs=2) as wp,
                tc.tile_pool(name="rteps", bufs=1, space="PSUM") as ps,
            ):
                fi_sb = wp.tile([128, 16, 4], F32, tag="fi_sb", bufs=1)
                lg = wp.tile([128, 16, E], F32, tag="lgall")
                nc.sync.dma_start(lg[:], lg_ag.rearrange("(p g) e -> p g e", p=128))
                mx = wp.tile([128, 16], F32, tag="mx")
                nc.vector.tensor_reduce(mx[:], lg[:], axis=AX.X, op=OP.max)
                sh_ = wp.tile([128, 16, E], F32, tag="shift")
                nc.vector.tensor_tensor(sh_[:], lg[:], mx[:, :, None].to_broadcast((128, 16, E)),
                                        op=OP.subtract)
                ex = wp.tile([128, 16, E], F32, tag="ex")
                nc.scalar.activation(ex[:], sh_[:], AF.Exp)
                sm = wp.tile([128, 16], F32, tag="sm")
                nc.vector.tensor_reduce(sm[:], ex[:], axis=AX.X, op=OP.add)
                rs = wp.tile([128, 16], F32, tag="rs")
                nc.vector.reciprocal(rs[:], sm[:])
                probs = wp.tile([128, 16, E], F32, tag="probs")
                nc.vector.tensor_tensor(probs[:], ex[:], rs[:, :, None].to_broadcast((128, 16, E)),
                                        op=OP.mult)
                zt = wp.tile([128, 16], F32, tag="zt")
                nc.scalar.activation(zt[:], sm[:], AF.Ln)
                nc.vector.tensor_tensor(zt[:], zt[:], mx[:], op=OP.add)
                nc.vector.tensor_tensor(zt[:], zt[:], zt[:], op=OP.mult)
                z2r = wp.tile([128, 1], F32, tag="z2r")
                nc.vector.tensor_reduce(z2r[:], zt[:], axis=AX.X, op=OP.add)
                z2p = ps.tile([1, 1], F32, tag="z2p")
                nc.tensor.matmul(z2p[:], ones_col[:], z2r[:], start=True, stop=True)
                ind1 = wp.tile([128, 16, E], F32, tag="ind1")
                nc.vector.tensor_tensor(ind1[:], lg[:], mx[:, :, None].to_broadcast((128, 16, E)),
                                        op=OP.is_ge)
                mp1 = wp.tile([128, 16], F32, tag="mp1")
                nc.vector.tensor_reduce(mp1[:], probs[:], axis=AX.X, op=OP.max)
                p2 = wp.tile([128, 16, E], F32, tag="p2")
                nc.vector.tensor_tensor(p2[:], ind1[:], probs[:], op=OP.mult)
                nc.vector.tensor_tensor(p2[:], probs[:], p2[:], op=OP.subtract)
                mp2 = wp.tile([128, 16], F32, tag="mp2")
                nc.vector.tensor_reduce(mp2[:], p2[:], axis=AX.X, op=OP.max)
                ind2 = wp.tile([128, 16, E], F32, tag="ind2")
                nc.vector.tensor_tensor(ind2[:], p2[:], mp2[:, :, None].to_broadcast((128, 16, E)),
                                        op=OP.is_ge)
                ind = wp.tile([128, 16, E], F32, tag="ind")
                nc.vector.tensor_tensor(ind[:], ind1[:], ind2[:], op=OP.add)
                wsum = wp.tile([128, 16], F32, tag="wsum")
                nc.vector.tensor_tensor(wsum[:], mp1[:], mp2[:], op=OP.add)
                nc.vector.reciprocal(wsum[:], wsum[:])
                w1 = wp.tile([128, 16], F32, tag="w1")
                w2 = wp.tile([128, 16], F32, tag="w2")
                nc.vector.tensor_tensor(w1[:], mp1[:], wsum[:], op=OP.mult)
                nc.vector.tensor_tensor(w2[:], mp2[:], wsum[:], op=OP.mult)
                totals = wp.tile([128, E], F32, tag="totals")
                nc.vector.tensor_reduce(totals[:], ind[:].rearrange("p g e -> p e g"),
                                        axis=AX.X, op=OP.add)
                probsum = wp.tile([128, E], F32, tag="probsum")
                nc.vector.tensor_reduce(probsum[:], probs[:].rearrange("p g e -> p e g"),
                                        axis=AX.X, op=OP.add)
                c0 = wp.tile([128, 16, E], F32, tag="c0")
                c1 = wp.tile([128, 16, E], F32, tag="c1")
                nc.vector.tensor_copy(c0[:], ind[:])
                src, dstc = c0, c1
                for shf in (1, 2, 4, 8):
                    nc.vector.tensor_copy(dstc[:, :shf, :], src[:, :shf, :])
                    nc.vector.tensor_tensor(dstc[:, shf:, :], src[:, shf:, :],
                                            src[:, :16 - shf, :], op=OP.add)
                    src, dstc = dstc, src
                incl = src
                excl = wp.tile([128, 16, E], F32, tag="excl")
                nc.vector.tensor_tensor(excl[:], incl[:], ind[:], op=OP.subtract)
                offp = ps.tile([128, E], F32, tag="offp")
                nc.tensor.matmul(offp[:], triu_sb[:], totals[:], start=True, stop=True)
                offs = wp.tile([128, E], F32, tag="offs")
                nc.vector.tensor_copy(offs[:], offp[:])
                pos = wp.tile([128, 16, E], F32, tag="pos")
                nc.vector.tensor_tensor(pos[:], excl[:], offs[:, None, :].to_broadcast((128, 16, E)),
                                        op=OP.add)
                keep = wp.tile([128, 16, E], F32, tag="keep")
                nc.vector.tensor_scalar(keep[:], pos[:], float(CAP), None, op0=OP.is_lt)
                indk = wp.tile([128, 16, E], F32, tag="indk")
                nc.vector.tensor_tensor(indk[:], ind[:], keep[:], op=OP.mult)
                mpos = wp.tile([128, 16, E], F32, tag="mpos")
                nc.vector.tensor_tensor(mpos[:], pos[:], indk[:], op=OP.mult)
                t9 = wp.tile([128, 16, E], F32, tag="t9")
                nc.vector.tensor_scalar(t9[:], indk[:], -9999.0, 9999.0, op0=OP.mult, op1=OP.add)
                nc.vector.tensor_tensor(mpos[:], mpos[:], t9[:], op=OP.add)
                me = wp.tile([128, 16, E], F32, tag="me")
                nc.vector.tensor_tensor(me[:], mpos[:],
                                        onehot_sb[:, None, :].to_broadcast((128, 16, E)), op=OP.mult)
                msl = wp.tile([128, 16], F32, tag="msl")
                nc.vector.tensor_reduce(msl[:], me[:], axis=AX.X, op=OP.add)
                ip1 = ps.tile([1, 512], F32, tag="ip1")
                ip2 = ps.tile([1, CAP - 512], F32, tag="ip2")
                for g in range(16):
                    tval = wp.tile([128, 1], F32, tag="tval")
                    nc.vector.tensor_scalar(tval[:], p16_sb[:], float(g), None, op0=OP.add)
                    eq = wp.tile([128, CAP], F32, tag="eq")
                    nc.vector.tensor_scalar(eq[:], slot_iota_sb[:], msl[:, g:g + 1], None,
                                            op0=OP.is_equal)
                    nc.tensor.matmul(ip1[:], tval[:], eq[:, 0:512], start=(g == 0), stop=(g == 15))
                    nc.tensor.matmul(ip2[:], tval[:], eq[:, 512:CAP], start=(g == 0), stop=(g == 15))
                idx_f = wp.tile([1, CAP], F32, tag="idx_f")
                nc.vector.tensor_copy(idx_f[:, 0:512], ip1[:])
                nc.vector.tensor_copy(idx_f[:, 512:CAP], ip2[:])
                idx_i = wp.tile([1, CAP], I32, tag="idx_i")
                nc.vector.tensor_copy(idx_i[:], idx_f[:])
                nc.sync.dma_start(idx_dram[None, :], idx_i[:])

                def build_fi(indx, wx, col_f, col_w):
                    ec = wp.tile([128, 16, E], F32, tag="ec")
                    nc.vector.tensor_tensor(ec[:], indx[:],
                                            eiota_sb[:, None, :].to_broadcast((128, 16, E)), op=OP.mult)
                    ev = wp.tile([128, 16], F32, tag="ev")
                    nc.vector.tensor_reduce(ev[:], ec[:], axis=AX.X, op=OP.add)
                    sc = wp.tile([128, 16, E], F32, tag="sc")
                    nc.vector.tensor_tensor(sc[:], indx[:], pos[:], op=OP.mult)
                    sv = wp.tile([128, 16], F32, tag="sv")
                    nc.vector.tensor_reduce(sv[:], sc[:], axis=AX.X, op=OP.add)
                    kc = wp.tile([128, 16, E], F32, tag="kc")
                    nc.vector.tensor_tensor(kc[:], indx[:], keep[:], op=OP.mult)
                    kv_ = wp.tile([128, 16], F32, tag="kv_")
                    nc.vector.tensor_reduce(kv_[:], kc[:], axis=AX.X, op=OP.add)
                    fl = wp.tile([128, 16], F32, tag="fl")
                    nc.vector.tensor_scalar(fl[:], ev[:], float(CAP), None, op0=OP.mult)
                    nc.vector.tensor_tensor(fl[:], fl[:], sv[:], op=OP.add)
                    nc.vector.tensor_scalar(fl[:], fl[:], float(E * CAP - 1), None, op0=OP.min)
                    nc.vector.tensor_copy(fi_sb[:, :, col_f], fl[:])
                    wv_t = wp.tile([128, 16], F32, tag="wv_t")
                    nc.vector.tensor_tensor(wv_t[:], wx[:], kv_[:], op=OP.mult)
                    nc.vector.tensor_copy(fi_sb[:, :, col_w], wv_t[:])

                build_fi(ind1, w1, 0, 1)
                build_fi(ind2, w2, 2, 3)
                nc.sync.dma_start(fi_dram.rearrange("(p g) c -> p (g c)", p=128),
                                  fi_sb[:].rearrange("p g c -> p (g c)"))
                cnt_p = ps.tile([1, E], F32, tag="cntp")
                nc.tensor.matmul(cnt_p[:], ones_col[:], totals[:], start=True, stop=True)
                ps_p = ps.tile([1, E], F32, tag="psp")
                nc.tensor.matmul(ps_p[:], ones_col[:], probsum[:], start=True, stop=True)
                st = wp.tile([1, 32], F32, tag="stats")
                nc.vector.memset(st[:], 0.0)
                nc.vector.tensor_copy(st[:, 0:E], cnt_p[:])
                nc.vector.tensor_copy(st[:, 8:8 + E], ps_p[:])
                nc.vector.tensor_copy(st[:, 16:17], z2p[:])
                nc.sync.dma_start(stats_out[:], st[:])

            # ---------- PHASE 6: expert FFN ----------
            with (
                tc.tile_pool(name="ex", bufs=2) as wp,
                tc.tile_pool(name="exps", bufs=2, space="PSUM") as ps,
                tc.tile_pool(name="exps_t", bufs=2, space="PSUM") as ps_t,
                tc.tile_pool(name="exps_y", bufs=1, space="PSUM") as ps_y,
            ):
                idx2 = wp.tile([128, 5], I32, tag="idx2")
                nc.sync.dma_start(idx2[:], idx_dram.rearrange("(g p) -> p g", p=128))
                exA = tc.alloc_tile_pool(name="exA", bufs=2)
                xbT = exA.tile([128, DT, CAP], F32R, tag="xbT", bufs=1)
                for gi in range(5):
                    xb_nat = exA.tile([128, D], mybir.dt.bfloat16, tag="xb_nat")
                    nc.gpsimd.indirect_dma_start(
                        out=xb_nat[:], out_offset=None, in_=u_ag[:],
                        in_offset=IndirectOffsetOnAxis(ap=idx2[:, gi:gi + 1], axis=0))
                    for dt in range(DT):
                        tp = ps_t.tile([128, 128], mybir.dt.bfloat16, tag="extp", bufs=1)
                        nc.tensor.transpose(tp[:], xb_nat[:, dt * 128:(dt + 1) * 128], ident_b[:])
                        nc.vector.tensor_scalar(xbT[:, dt, gi * 128:(gi + 1) * 128], tp[:],
                                                ln2_col[:, dt:dt + 1], None, op0=OP.mult)
                heT = wp.tile([128, MT, CAP], F32R, tag="heT", bufs=1)
                for mt in range(MT):
                    wg = exA.tile([128, DT, 128], F32R, tag="wge", bufs=2)
                    nc.sync.dma_start(wg[:], wge_h[mt])
                    wu = exA.tile([128, DT, 128], F32R, tag="wue", bufs=2)
                    nc.sync.dma_start(wu[:], wue_h[mt])
                    for ch in range(2):
                        cs = slice(ch * 320, (ch + 1) * 320)
                        gp = ps.tile([128, 320], F32, tag="gep")
                        up = ps.tile([128, 320], F32, tag="uep")
                        for dt in range(DT):
                            nc.tensor.matmul(gp[:], wg[:, dt, :], xbT[:, dt, cs],
                                             start=(dt == 0), stop=(dt == DT - 1))
                            nc.tensor.matmul(up[:], wu[:, dt, :], xbT[:, dt, cs],
                                             start=(dt == 0), stop=(dt == DT - 1))
                        sg = wp.tile([128, 320], F32, tag="sge")
                        nc.scalar.activation(sg[:], gp[:], AF.Silu)
                        nc.vector.tensor_tensor(heT[:, mt, cs], sg[:], up[:], op=OP.mult)
                exA.release()
                exB = tc.alloc_tile_pool(name="exB", bufs=2)
                yb_nat = exB.tile([128, 5, D], mybir.dt.bfloat16, tag="yb_nat", bufs=1)
                for dt in range(DT):
                    wd = exB.tile([128, MT, 128], F32R, tag="wde", bufs=2)
                    nc.sync.dma_start(wd[:], wde_h[dt])
                    ypA = ps_y.tile([128, 320], F32, tag="ydpA")
                    ypB = ps_y.tile([128, 320], F32, tag="ydpB")
                    for mt in range(MT):
                        nc.tensor.matmul(ypA[:], wd[:, mt, :], heT[:, mt, 0:320],
                                         start=(mt == 0), stop=(mt == MT - 1))
                        nc.tensor.matmul(ypB[:], wd[:, mt, :], heT[:, mt, 320:CAP],
                                         start=(mt == 0), stop=(mt == MT - 1))
                    ys = exB.tile([128, CAP], F32R, tag="ys")
                    nc.vector.tensor_copy(ys[:, 0:320], ypA[:])
                    nc.vector.tensor_copy(ys[:, 320:CAP], ypB[:])
                    for gi in range(5):
                        tp = ps_t.tile([128, 128], F32R, tag="extpr", bufs=1)
                        nc.tensor.transpose(tp[:], ys[:, gi * 128:(gi + 1) * 128], ident_r[:])
                        nc.vector.tensor_copy(yb_nat[:, gi, dt * 128:(dt + 1) * 128], tp[:])
                for gi in range(5):
                    nc.sync.dma_start(yb_loc[gi * 128:(gi + 1) * 128, :], yb_nat[:, gi, :])
                nc.gpsimd.collective_compute("AllGather", OP.bypass, replica_groups=RG,
                                             ins=[yb_loc[:]], outs=[yb_ag[:]])
                exB.release()

            # (shared part B emitted after expert AG)
            shared_gate_up(1)
            shared_down()

            # ---------- PHASE 7: combine ----------
            with tc.tile_pool(name="cb", bufs=2) as wp:
                fi_my = wp.tile([128, 2, 4], F32, tag="fi_my")
                for tt in range(2):
                    nc.gpsimd.indirect_dma_start(
                        out=fi_my[:, tt, :], out_offset=None, in_=fi_dram[:],
                        in_offset=IndirectOffsetOnAxis(ap=own_rows_sb[:, tt:tt + 1], axis=0))
                of1 = wp.tile([128, 2], I32, tag="of1")
                of2 = wp.tile([128, 2], I32, tag="of2")
                nc.vector.tensor_copy(of1[:], fi_my[:, :, 0])
                nc.vector.tensor_copy(of2[:], fi_my[:, :, 2])
                for tt in range(2):
                    g1 = wp.tile([128, D], mybir.dt.bfloat16, tag="g1")
                    g2 = wp.tile([128, D], mybir.dt.bfloat16, tag="g2")
                    nc.gpsimd.indirect_dma_start(
                        out=g1[:], out_offset=None, in_=yb_ag[:],
                        in_offset=IndirectOffsetOnAxis(ap=of1[:, tt:tt + 1], axis=0))
                    nc.gpsimd.indirect_dma_start(
                        out=g2[:], out_offset=None, in_=yb_ag[:],
                        in_offset=IndirectOffsetOnAxis(ap=of2[:, tt:tt + 1], axis=0))
                    moe = wp.tile([128, D], F32, tag="moe")
                    moe2 = wp.tile([128, D], F32, tag="moe2")
                    nc.vector.tensor_scalar(moe[:], g1[:], fi_my[:, tt, 1:2], None, op0=OP.mult)
                    nc.vector.tensor_scalar(moe2[:], g2[:], fi_my[:, tt, 3:4], None, op0=OP.mult)
                    nc.vector.tensor_tensor(moe[:], moe[:], moe2[:], op=OP.add)
                    yt = wp.tile([128, D], F32, tag="yt")
                    nc.vector.tensor_tensor(yt[:], h_sb[:, tt, :], shared_nat[:, tt, :], op=OP.add)
                    nc.vector.tensor_tensor(yt[:], yt[:], moe[:], op=OP.add)
                    nc.sync.dma_start(y_out[tt * 128:(tt + 1) * 128, :], yt[:])
            shp.release()
            uTp.release()
            uTrp.release()

    nc.compile()
    return nc


_NC_CACHE = None


def _get_program():
    global _NC_CACHE
    if _NC_CACHE is None:
        _NC_CACHE = build_program()
    return _NC_CACHE


def _host_tables():
    inv_freq = 1.0 / THETA ** (np.arange(HALF, dtype=np.float32) / HALF)
    ang = np.arange(S, dtype=np.float32)[:, None] * inv_freq[None, :]
    cos_full = np.concatenate([np.cos(ang).T, np.cos(ang).T]).astype(np.float32)   # [64, S]
    ssin_full = np.concatenate([-np.sin(ang).T, np.sin(ang).T]).astype(np.float32)
    perm_to_true = np.empty(T, dtype=np.int64)
    for s in range(NB):
        tb = true_block(s)
        perm_to_true[s * 128:(s + 1) * 128] = np.arange(tb * 128, tb * 128 + 128)
    triu = np.triu(np.ones((128, 128), np.float32), 1)
    # swap64[k, m] = 1 iff m = (k+32) % 64  (out = swap64.T @ x swaps halves)
    swap64 = np.zeros((ROT, ROT), np.float32)
    for k_ in range(ROT):
        swap64[k_, (k_ + HALF) % ROT] = 1.0
    slot_iota = np.broadcast_to(np.arange(CAP, dtype=np.float32), (128, CAP)).copy()
    p16 = (np.arange(128, dtype=np.float32) * 16).reshape(128, 1)
    e_iota = np.broadcast_to(np.arange(E, dtype=np.float32), (128, E)).copy()
    kpos_cols = np.empty((128, NB), np.float32)
    for s in range(NB):
        kpos_cols[:, s] = perm_to_true[s * 128:(s + 1) * 128]
    return cos_full, ssin_full, perm_to_true, triu, slot_iota, p16, e_iota, kpos_cols, swap64


def _col_major(w, n_outer, q=128):
    """[D_in, n_outer*q] -> [n_outer, 128, D_in//128, q] with
    (o, p, t, c) = w[t*128+p, o*q+c]; contiguous per (o, p)."""
    d_in = w.shape[0]
    return np.ascontiguousarray(
        w.reshape(d_in // 128, 128, n_outer, q).transpose(2, 1, 0, 3))


def kernel(x, ln1_w, ln2_w, w_q, w_k, w_v, w_o, attn_gate, w_router,
           w_gate_e, w_up_e, w_down_e, w_gate_s, w_up_s, w_down_s,
           _trace=False):
    nc = _get_program()
    (cos_full, ssin_full, perm_to_true, triu, slot_iota, p16, e_iota, kpos_cols,
     swap64) = _host_tables()

    f32 = lambda a: np.ascontiguousarray(np.asarray(a, dtype=np.float32))
    x2 = f32(x).reshape(T, D)
    w_gate_e, w_up_e, w_down_e = f32(w_gate_e), f32(w_up_e), f32(w_down_e)
    shared_inputs = dict(
        x_nat=None,
        w_q=f32(w_q), w_k=f32(w_k),
        w_v=f32(w_v), w_o=f32(w_o),
        attn_gate=f32(attn_gate),
        wr_h=np.ascontiguousarray(f32(w_router).reshape(DT, 128, E).transpose(1, 0, 2)),
        w_gs=f32(w_gate_s), w_us=f32(w_up_s), w_ds=f32(w_down_s),
        ln1_w=f32(ln1_w), ln2_w=f32(ln2_w),
        triu=triu, slot_iota=slot_iota, p16_col=p16, e_iota=e_iota,
        kpos_cols=kpos_cols, swap64_t=swap64,
    )
    in_maps = []
    for c in range(NC):
        rows = perm_to_true[c * TPC:(c + 1) * TPC]
        m = dict(shared_inputs)
        m["x_nat"] = np.ascontiguousarray(x2[rows])
        m["wge_h"] = _col_major(w_gate_e[c], MT)
        m["wue_h"] = _col_major(w_up_e[c], MT)
        m["wde_h"] = _col_major(w_down_e[c], DT)
        m["cos2_t"] = np.ascontiguousarray(cos_full[:, rows])
        m["ssin2_t"] = np.ascontiguousarray(ssin_full[:, rows])
        m["qpos_bc"] = np.ascontiguousarray(
            np.broadcast_to(rows.astype(np.float32), (128, TPC)))
        m["onehot_e"] = np.broadcast_to(
            (np.arange(E) == c).astype(np.float32), (128, E)).copy()
        m["own_rows"] = np.ascontiguousarray(
            (c * TPC + np.arange(TPC, dtype=np.int32)).reshape(2, 128).T)
        in_maps.append(m)

    res = run_bass_kernel_spmd(nc, in_maps, core_ids=list(range(NC)), trace=_trace)

    y = np.empty((T, D), np.float32)
    for c in range(NC):
        rows = perm_to_true[c * TPC:(c + 1) * TPC]
        y[rows] = res.results[c]["y_out"]
    st = res.results[0]["stats_out"][0]
    counts, probsum, z2 = st[0:E], st[8:8 + E], st[16]
    frac = counts / max(float(counts.sum()), 1.0)
    mean_probs = probsum / T
    lb_loss = E * float((frac * K * mean_probs).sum())
    z_loss = float(z2) / T
    aux = np.float32(0.01 * lb_loss + 0.001 * z_loss)
    out = (y.reshape(B, S, D), aux)
    if _trace:
        return out, res
    return out


# revision 29
# speedup vs baseline: 1.1081x; 1.0015x over previous
"""Trainium2 Bass kernel for nn_Block_78280073937290 (moe_routing).

8-core SPMD plan:
- Token-parallel attention; core c owns true 128-token blocks {c, 15-c}
  (causal-balanced).  All per-core variation is input DATA (the program is
  identical on every core).
- fp32r (tf32-precision, full-rate) matmuls on the attention/shared/expert
  paths; true fp32 matmuls for router logits (top-2 selection is
  precision-critical) and integer-valued index builds.
- MoE: router logits AllGathered; routing/slots computed redundantly on all
  cores from identical fp32 logits; each core indirect-gathers the tokens
  routed to its expert from the AllGathered u, runs the expert FFN, AllGathers
  the compact outputs, and combines its own tokens' two expert rows locally.
- aux-loss partials are computed on device; the scalar is assembled on host.
"""

import numpy as np

import concourse.bass as bass
import concourse.mybir as mybir
import concourse.tile as tile
from concourse import bacc
from concourse.bass import IndirectOffsetOnAxis
from concourse.bass_utils import run_bass_kernel_spmd
from concourse.masks import make_identity

F32 = mybir.dt.float32
F32R = mybir.dt.float32r
I32 = mybir.dt.int32
AF = mybir.ActivationFunctionType
OP = mybir.AluOpType
AX = mybir.AxisListType

B, S, D = 1, 2048, 2048
N, MKV, HD = 16, 4, 128
E, K = 8, 2
MI, MS = 1408, 1408
G = 12
ROT = HD // 2          # 64
HALF = ROT // 2        # 32
THETA = 1024.0
CAP = int(B * S * K / E * 1.25)  # 640
EPS = 1e-5
T = B * S              # 2048
NC = 8
TPC = T // NC          # 256
NB = S // 128          # 16
DT = D // 128          # 16
MT = MI // 128         # 11
INV_SQRT_HD = 1.0 / float(np.sqrt(HD))


def true_block(s):
    return s // 2 if s % 2 == 0 else 15 - s // 2


def r32(ap):
    return ap.bitcast(F32R)


def build_program():
    nc = bacc.Bacc("TRN2", target_bir_lowering=False, debug=False, num_devices=NC)

    def inp(name, shape, dtype=F32):
        return nc.dram_tensor(name, shape, dtype, kind="ExternalInput").ap()

    def inp_r(name, shape):
        return nc.dram_tensor(name, shape, F32R, kind="ExternalInput").ap()

    # activations / tables (per-core data)
    x_nat = inp("x_nat", [TPC, D])
    cos2_t = inp("cos2_t", [ROT, TPC])
    ssin2_t = inp("ssin2_t", [ROT, TPC])
    qpos_bc = inp("qpos_bc", [128, TPC])
    kpos_cols = inp("kpos_cols", [128, NB])
    triu = inp("triu", [128, 128])
    swap64_t = inp_r("swap64_t", [ROT, ROT])
    slot_iota = inp("slot_iota", [128, CAP])
    p16_col = inp("p16_col", [128, 1])
    onehot_e = inp("onehot_e", [128, E])
    e_iota = inp("e_iota", [128, E])
    own_rows = inp("own_rows", [128, 2], I32)
    ln1_w = inp("ln1_w", [D])
    ln2_w = inp("ln2_w", [D])
    attn_gate = inp_r("attn_gate", [G, N])
    # weights, host pre-laid-out for contiguous per-partition DMA
    w_q = inp_r("w_q", [D, N * HD])              # natural
    w_k = inp_r("w_k", [D, MKV * HD])
    w_v = inp_r("w_v", [D, MKV * HD])
    w_o = inp_r("w_o", [N * HD, D])
    w_router = inp("wr_h", [128, DT, E])       # (p,dt,e) = w_router[dt*128+p, e]
    w_gs = inp_r("w_gs", [D, MS])
    w_us = inp_r("w_us", [D, MS])
    w_ds = inp_r("w_ds", [MS, D])
    wge_h = inp_r("wge_h", [MT, 128, DT, 128])
    wue_h = inp_r("wue_h", [MT, 128, DT, 128])
    wde_h = inp_r("wde_h", [DT, 128, MT, 128])

    y_out = nc.dram_tensor("y_out", [TPC, D], F32, kind="ExternalOutput").ap()
    stats_out = nc.dram_tensor("stats_out", [1, 32], F32, kind="ExternalOutput").ap()
    dbg_logits = nc.dram_tensor("dbg_logits", [T, E], F32, kind="ExternalOutput").ap()

    kT_loc = nc.dram_tensor("kT_loc", [MKV, 128, TPC], F32R).ap()
    kT_ag = nc.dram_tensor("kT_ag", [NC * MKV, 128, TPC], F32R, addr_space="Shared").ap()
    # v stored partition-major per kv head: (kvh, p, tt, hd)
    v_loc = nc.dram_tensor("v_loc", [MKV, 128, 2, HD], F32R).ap()
    v_ag = nc.dram_tensor("v_ag", [NC * MKV, 128, 2, HD], F32R, addr_space="Shared").ap()
    u_loc = nc.dram_tensor("u_loc", [TPC, D], mybir.dt.bfloat16).ap()
    u_ag = nc.dram_tensor("u_ag", [T, D], mybir.dt.bfloat16, addr_space="Shared").ap()
    lg_loc = nc.dram_tensor("lg_loc", [TPC, E], F32).ap()
    lg_ag = nc.dram_tensor("lg_ag", [T, E], F32, addr_space="Shared").ap()
    yb_loc = nc.dram_tensor("yb_loc", [CAP, D], mybir.dt.bfloat16).ap()
    yb_ag = nc.dram_tensor("yb_ag", [E * CAP, D], mybir.dt.bfloat16, addr_space="Shared").ap()
    idx_dram = nc.dram_tensor("idx_dram", [CAP], I32).ap()
    fi_dram = nc.dram_tensor("fi_dram", [T, 4], F32).ap()

    RG = [list(range(NC))]

    with tile.TileContext(nc) as tc:
        with (
            tc.tile_pool(name="persist", bufs=1) as pp,
            tc.tile_pool(name="const", bufs=1) as cp,
        ):
            ident = cp.tile([128, 128], F32, tag="ident")
            make_identity(nc, ident)
            ident_r = cp.tile([128, 128], F32R, tag="ident_r")
            ones_col_r = cp.tile([128, 1], F32R, tag="ones_col_r")
            ones_col = cp.tile([128, 1], F32, tag="ones_col")
            nc.vector.memset(ones_col[:], 1.0)
            ones_row = cp.tile([1, 128], F32, tag="ones_row")
            nc.vector.memset(ones_row[:], 1.0)
            nc.vector.tensor_copy(ident_r[:], ident[:])
            ident_b = cp.tile([128, 128], mybir.dt.bfloat16, tag="ident_b")
            nc.vector.tensor_copy(ident_b[:], ident[:])
            swap64 = cp.tile([ROT, ROT], F32R, tag="swap64")
            nc.sync.dma_start(swap64[:], swap64_t)
            nc.vector.tensor_copy(ones_col_r[:], ones_col[:])
            eps_ln = cp.tile([128, 1], F32, tag="eps_ln")
            nc.vector.memset(eps_ln[:], EPS)
            eps_qk = cp.tile([1, 1], F32, tag="eps_qk")
            nc.vector.memset(eps_qk[:], 1e-6)

            def load_const(ap_dram, shape, tag, dtype=F32):
                t = cp.tile(shape, dtype, tag=tag)
                nc.sync.dma_start(t[:], ap_dram)
                return t

            cos2_sb = load_const(cos2_t, [ROT, TPC], "cos2")
            ssin2_sb = load_const(ssin2_t, [ROT, TPC], "ssin2")
            qpos_sb = load_const(qpos_bc, [128, TPC], "qpos")
            kpos_sb = load_const(kpos_cols, [128, NB], "kpos")
            triu_sb = load_const(triu, [128, 128], "triu")
            slot_iota_sb = load_const(slot_iota, [128, CAP], "slot_iota")
            p16_sb = load_const(p16_col, [128, 1], "p16")
            onehot_sb = load_const(onehot_e, [128, E], "onehot")
            eiota_sb = load_const(e_iota, [128, E], "eiota")
            own_rows_sb = load_const(own_rows, [128, 2], "own_rows", I32)
            ln1_col = load_const(ln1_w.rearrange("(o p) -> p o", p=128), [128, DT], "ln1col")
            ln2_col = load_const(ln2_w.rearrange("(o p) -> p o", p=128), [128, DT], "ln2col")
            ag_sb = load_const(attn_gate, [G, N], "ag", F32R)

            h_sb = pp.tile([128, 2, D], F32, tag="h_sb")

            uTrp = tc.alloc_tile_pool(name="uTrp", bufs=1)
            uTr = uTrp.tile([128, DT, TPC], F32R, tag="uTr")
            uTp = tc.alloc_tile_pool(name="uTp", bufs=1)
            uT = uTp.tile([128, DT, TPC], F32, tag="uT")
            actp = tc.alloc_tile_pool(name="actp", bufs=1)
            u1T = actp.tile([128, DT, TPC], F32R, tag="u1T")
            qT = actp.tile([128, N, TPC], F32R, tag="qT")
            oT = actp.tile([128, N, TPC], F32R, tag="oT")
            gZ = actp.tile([1, N, TPC], F32, tag="gZ")

            # ---------- PHASE 0: u1 = rmsnorm(x); u1T ----------
            with (
                tc.tile_pool(name="ph0", bufs=2) as wp,
                tc.tile_pool(name="ph0ps", bufs=2, space="PSUM") as ps,
            ):
                u1_nat = wp.tile([128, 2, D], F32, tag="u1nat", bufs=1)
                x_ph0 = wp.tile([128, 2, D], F32, tag="x_ph0", bufs=1)
                for tt in range(2):
                    nc.sync.dma_start(x_ph0[:, tt, :], x_nat[tt * 128:(tt + 1) * 128, :])
                for tt in range(2):
                    sq = wp.tile([128, D], F32, tag="sq")
                    nc.vector.tensor_tensor(sq[:], x_ph0[:, tt, :], x_ph0[:, tt, :], op=OP.mult)
                    ssum = wp.tile([128, 1], F32, tag="ssum")
                    nc.vector.tensor_reduce(ssum[:], sq[:], axis=AX.X, op=OP.add)
                    rt = wp.tile([128, 1], F32, tag="rt")
                    nc.scalar.activation(rt[:], ssum[:], AF.Sqrt, scale=1.0 / D, bias=eps_ln[:])
                    nc.vector.reciprocal(rt[:], rt[:])
                    nc.vector.tensor_scalar(u1_nat[:, tt, :], x_ph0[:, tt, :], rt[:], None, op0=OP.mult)
                for dt in range(DT):
                    for tt in range(2):
                        pt = ps.tile([128, 128], F32, tag="tp")
                        nc.tensor.transpose(pt[:], u1_nat[:, tt, dt * 128:(dt + 1) * 128], ident[:])
                        nc.vector.tensor_scalar(u1T[:, dt, tt * 128:(tt + 1) * 128], pt[:],
                                                ln1_col[:, dt:dt + 1], None, op0=OP.mult)

            # ---------- PHASE 1: QKV (natural, N=512 moving), norms, rope ----------
            with (
                tc.tile_pool(name="ph1", bufs=2) as wp,
                tc.tile_pool(name="ph1r", bufs=1) as rp,
                tc.tile_pool(name="ph1w", bufs=2) as wgt,
            ):
                rq_flat = rp.tile([1, N, TPC], F32, tag="rq_flat")
                rk_flat = rp.tile([1, MKV, TPC], F32, tag="rk_flat")
                wk_r = w_k.rearrange("(dt p) m -> dt p m", p=128)
                wv_r = w_v.rearrange("(dt p) m -> dt p m", p=128)
                wq_r = w_q.rearrange("(dt p) m -> dt p m", p=128)

                # --- pass 1: k_nat, v_nat ---
                ps_kv = tc.alloc_tile_pool(name="ps_kv", bufs=1, space="PSUM")
                pk = [ps_kv.tile([128, MKV * HD], F32, tag=f"pk{tt}", name=f"pk{tt}")
                      for tt in range(2)]
                pv = [ps_kv.tile([128, MKV * HD], F32, tag=f"pv{tt}", name=f"pv{tt}")
                      for tt in range(2)]
                for dt in range(DT):
                    wtk = wgt.tile([128, MKV * HD], F32R, tag="wtk")
                    nc.sync.dma_start(wtk[:], wk_r[dt])
                    wtv = wgt.tile([128, MKV * HD], F32R, tag="wtv")
                    nc.sync.dma_start(wtv[:], wv_r[dt])
                    for tt in range(2):
                        nc.tensor.matmul(pk[tt][:], u1T[:, dt, tt * 128:(tt + 1) * 128],
                                         wtk[:], start=(dt == 0), stop=(dt == DT - 1))
                        nc.tensor.matmul(pv[tt][:], u1T[:, dt, tt * 128:(tt + 1) * 128],
                                         wtv[:], start=(dt == 0), stop=(dt == DT - 1))
                k_nat = rp.tile([128, 2, MKV * HD], F32R, tag="k_nat")
                v_sb = rp.tile([128, 2, MKV * HD], F32R, tag="v_sb")
                for tt in range(2):
                    nc.vector.tensor_copy(k_nat[:, tt, :], pk[tt][:])
                    nc.vector.tensor_copy(v_sb[:, tt, :], pv[tt][:])
                ps_kv.release()

                ps_m1 = tc.alloc_tile_pool(name="ps_m1", bufs=1, space="PSUM")
                for kh in range(MKV):
                    for tt in range(2):
                        nc.sync.dma_start(v_loc[kh, :, tt, :], v_sb[:, tt, kh * HD:(kh + 1) * HD])
                nc.gpsimd.collective_compute("AllGather", OP.bypass, replica_groups=RG,
                                             ins=[v_loc[:]], outs=[v_ag[:]])

                def qknorm_rope(dst, rdst, wp=wp, ps1=None):
                    """dst: F32R sbuf [128, TPC] (in place); rdst [1, TPC] @p0."""
                    sq = wp.tile([128, TPC], F32, tag="sqh")
                    nc.vector.tensor_tensor(sq[:], dst, dst, op=OP.mult)
                    sp = ps1.tile([1, TPC], F32, tag="normsum", bufs=1)
                    nc.tensor.matmul(sp[:], ones_col[:], sq[:], start=True, stop=True)
                    nc.scalar.activation(rdst, sp[:], AF.Sqrt, scale=1.0 / HD, bias=eps_qk[:])
                    nc.vector.reciprocal(rdst, rdst)
                    xsp = ps1.tile([ROT, TPC], F32, tag="xsp", bufs=2)
                    nc.tensor.matmul(xsp[:], swap64[:], dst[0:ROT, :], start=True, stop=True)
                    tmp = wp.tile([ROT, TPC], F32, tag="tmp_rope")
                    nc.vector.tensor_tensor(tmp[:], xsp[:], ssin2_sb[:], op=OP.mult)
                    xc = wp.tile([ROT, TPC], F32, tag="xc")
                    nc.vector.tensor_tensor(xc[:], dst[0:ROT, :], cos2_sb[:], op=OP.mult)
                    nc.vector.tensor_tensor(dst[0:ROT, :], xc[:], tmp[:], op=OP.add)

                def fold_scale(dst, row_ap, psx):
                    bp = psx.tile([128, TPC], F32, tag="bcast", bufs=2)
                    nc.tensor.matmul(bp[:], ones_row[:], row_ap, start=True, stop=True)
                    bs = wp.tile([128, TPC], F32, tag="bcast_sb")
                    nc.scalar.copy(bs[:], bp[:])
                    nc.vector.tensor_tensor(dst, dst, bs[:], op=OP.mult)

                kT_sb = rp.tile([128, MKV, TPC], F32R, tag="kT_sb")
                for kh in range(MKV):
                    for tt in range(2):
                        ktp = ps_m1.tile([128, 128], F32R, tag="ktp", bufs=2)
                        nc.tensor.transpose(ktp[:], k_nat[:, tt, kh * 128:(kh + 1) * 128],
                                            ident_r[:])
                        nc.scalar.copy(kT_sb[:, kh, tt * 128:(tt + 1) * 128], ktp[:])
                    qknorm_rope(kT_sb[:, kh, :], rk_flat[:, kh, :], ps1=ps_m1)
                    fold_scale(kT_sb[:, kh, :], rk_flat[:, kh, :], ps_m1)
                    nc.sync.dma_start(kT_loc[kh], kT_sb[:, kh, :])
                nc.gpsimd.collective_compute("AllGather", OP.bypass, replica_groups=RG,
                                             ins=[kT_loc[:]], outs=[kT_ag[:]])
                ps_m1.release()

                # --- pass 2: q_nat ---
                ps_q = tc.alloc_tile_pool(name="ps_q", bufs=1, space="PSUM")
                pq = [[ps_q.tile([128, 512], F32, tag=f"pq{tt}{ch}", name=f"pq{tt}{ch}")
                       for ch in range(4)] for tt in range(2)]
                for dt in range(DT):
                    wtq = wgt.tile([128, N * HD], F32R, tag="wtq")
                    nc.sync.dma_start(wtq[:], wq_r[dt])
                    for tt in range(2):
                        for ch in range(4):
                            nc.tensor.matmul(pq[tt][ch][:],
                                             u1T[:, dt, tt * 128:(tt + 1) * 128],
                                             wtq[:, ch * 512:(ch + 1) * 512],
                                             start=(dt == 0), stop=(dt == DT - 1))
                q_nat = rp.tile([128, 2, N * HD], F32R, tag="q_nat")
                for tt in range(2):
                    for ch in range(4):
                        nc.vector.tensor_copy(q_nat[:, tt, ch * 512:(ch + 1) * 512],
                                              pq[tt][ch][:])
                ps_q.release()

                ps_m2 = tc.alloc_tile_pool(name="ps_m2", bufs=1, space="PSUM")
                for h in range(N):
                    for tt in range(2):
                        qtp = ps_m2.tile([128, 128], F32R, tag="qtp", bufs=2)
                        nc.tensor.transpose(qtp[:], q_nat[:, tt, h * 128:(h + 1) * 128],
                                            ident_r[:])
                        nc.scalar.copy(qT[:, h, tt * 128:(tt + 1) * 128], qtp[:])
                    qknorm_rope(qT[:, h, :], rq_flat[:, h, :], ps1=ps_m2)
                    fold_scale(qT[:, h, :], rq_flat[:, h, :], ps_m2)
                # attention output gate
                for h in range(N):
                    gp = ps_m2.tile([1, TPC], F32, tag="gTp", bufs=1)
                    nc.tensor.matmul(gp[:], ag_sb[:, h:h + 1], u1T[0:G, 0, :],
                                     start=True, stop=True)
                    nc.scalar.activation(gZ[:, h, :], gp[:], AF.Sigmoid)
                nc.vector.tensor_scalar(gZ[:].rearrange("o h t -> o (h t)"),
                                        gZ[:].rearrange("o h t -> o (h t)"),
                                        2.0, None, op0=OP.mult)
                ps_m2.release()

            # ---------- PHASE 2: attention core ----------
            with (
                tc.tile_pool(name="att", bufs=2) as wp,
                tc.tile_pool(name="attkv", bufs=1) as kvp,
                tc.tile_pool(name="attpt", bufs=16) as ptp,
                tc.tile_pool(name="attps_s", bufs=2, space="PSUM") as ps_s,
                tc.tile_pool(name="attps_t", bufs=2, space="PSUM") as ps_t,
                tc.tile_pool(name="attps_o", bufs=2, space="PSUM") as ps_o,
                tc.tile_pool(name="attps_z", bufs=1, space="PSUM") as ps_z,
                tc.tile_pool(name="attps_b", bufs=1, space="PSUM") as ps_b,
            ):
                m01T = kvp.tile([128, NB, TPC], mybir.dt.bfloat16, tag="m01T")
                for s in range(NB):
                    nc.vector.tensor_scalar(m01T[:, s, :], qpos_sb[:], kpos_sb[:, s:s + 1],
                                            None, op0=OP.is_ge)
                for kvh in range(MKV):
                    kT_h = kvp.tile([128, NB * 128], F32R, tag="kT_h")
                    for c in range(NC):
                        nc.sync.dma_start(kT_h[:, c * 256:(c + 1) * 256],
                                          kT_ag[c * MKV + kvh, :, :])
                    v_h = kvp.tile([128, NB, HD], F32R, tag="v_h")
                    for c in range(NC):
                        nc.sync.dma_start(v_h[:, 2 * c:2 * c + 2, :],
                                          v_ag[c * MKV + kvh, :, :, :])
                    kT_h3 = kT_h[:].rearrange("p (s q) -> p s q", q=128)
                    for qh in range(4):
                        h = kvh * 4 + qh
                        probsT = [ptp.tile([128, TPC], F32R, tag="probsT", name=f"probsT{h}_{si}")
                                  for si in range(NB)]
                        for qb in range(2):
                            nslots = 8 if qb == 0 else NB
                            pt_buf = wp.tile([128, NB * 128], F32R, tag="ptbuf", bufs=1)
                            nmm = 2 if qb == 0 else 4
                            for m in range(nmm):
                                sp = ps_s.tile([128, 512], F32, tag="scores")
                                if qb == 0:
                                    rhs = kT_h3[:, ::2, :][:, m * 4:(m + 1) * 4, :]
                                else:
                                    rhs = kT_h[:, m * 512:(m + 1) * 512]
                                nc.tensor.matmul(sp[:], qT[:, h, qb * 128:(qb + 1) * 128],
                                                 rhs, start=True, stop=True)
                                nc.scalar.activation(pt_buf[:, m * 512:(m + 1) * 512], sp[:],
                                                     AF.Exp, scale=INV_SQRT_HD)
                            for i in range(nslots):
                                s = 2 * i if qb == 0 else i
                                tp = ps_t.tile([128, 128], F32R, tag="trp")
                                nc.tensor.transpose(tp[:], pt_buf[:, i * 128:(i + 1) * 128], ident_r[:])
                                nc.vector.tensor_tensor(
                                    probsT[s][:, qb * 128:(qb + 1) * 128], tp[:],
                                    m01T[:, s, qb * 128:(qb + 1) * 128], op=OP.mult)
                        zp = ps_z.tile([1, TPC], F32, tag="z")
                        op_ = ps_o.tile([128, TPC], F32, tag="oTp")
                        for s in range(NB):
                            if s % 2 == 0:
                                rhs, zsl, osl = probsT[s][:], zp[:], op_[:]
                            else:
                                rhs = probsT[s][:, 128:TPC]
                                zsl, osl = zp[:, 128:TPC], op_[:, 128:TPC]
                            first, last = s == 0, s == NB - 1
                            nc.tensor.matmul(zsl, ones_col_r[:], rhs, start=first, stop=last)
                            nc.tensor.matmul(osl, v_h[:, s, :], rhs, start=first, stop=last)
                        srow = wp.tile([1, TPC], F32, tag="srow")
                        nc.vector.reciprocal(srow[:], zp[:])
                        nc.vector.tensor_tensor(srow[:], srow[:], gZ[:, h, :], op=OP.mult)
                        bp = ps_b.tile([128, TPC], F32, tag="sb_bc")
                        nc.tensor.matmul(bp[:], ones_row[:], srow[:], start=True, stop=True)
                        bs = wp.tile([128, TPC], F32, tag="sb_bc_sb")
                        nc.vector.tensor_copy(bs[:], bp[:])
                        nc.vector.tensor_tensor(oT[:, h, :], op_[:], bs[:], op=OP.mult)

            # ---------- PHASE 2b: w_o, residual, ln2, uT ----------
            with (
                tc.tile_pool(name="wo", bufs=2) as wgt,
                tc.tile_pool(name="wops", bufs=1, space="PSUM") as ps,
            ):
                wo_r = w_o.rearrange("(h p) d -> h p d", p=128)
                x_rb = wgt.tile([128, 2, D], F32, tag="x_rb", bufs=1)
                for tt in range(2):
                    nc.sync.dma_start(x_rb[:, tt, :], x_nat[tt * 128:(tt + 1) * 128, :])
                pgrid = [[ps.tile([128, 512], F32, tag=f"wop{qb}{ch}", name=f"wop{qb}{ch}")
                          for ch in range(4)] for qb in range(2)]
                for h in range(N):
                    wo_h = wgt.tile([128, D], F32R, tag="wo_h")
                    nc.sync.dma_start(wo_h[:], wo_r[h])
                    for qb in range(2):
                        for ch in range(4):
                            nc.tensor.matmul(pgrid[qb][ch][:],
                                             oT[:, h, qb * 128:(qb + 1) * 128],
                                             wo_h[:, ch * 512:(ch + 1) * 512],
                                             start=(h == 0), stop=(h == N - 1))
                for qb in range(2):
                    for ch in range(4):
                        nc.vector.tensor_tensor(h_sb[:, qb, ch * 512:(ch + 1) * 512],
                                                pgrid[qb][ch][:],
                                                x_rb[:, qb, ch * 512:(ch + 1) * 512], op=OP.add)
            with (
                tc.tile_pool(name="ln2", bufs=2) as wp,
                tc.tile_pool(name="ln2ps", bufs=2, space="PSUM") as ps,
            ):
                u_sb = wp.tile([128, 2, D], F32, tag="u_sb", bufs=1)
                for tt in range(2):
                    sq = wp.tile([128, D], F32, tag="sq2")
                    nc.vector.tensor_tensor(sq[:], h_sb[:, tt, :], h_sb[:, tt, :], op=OP.mult)
                    ssum = wp.tile([128, 1], F32, tag="ssum2")
                    nc.vector.tensor_reduce(ssum[:], sq[:], axis=AX.X, op=OP.add)
                    rt = wp.tile([128, 1], F32, tag="rt2")
                    nc.scalar.activation(rt[:], ssum[:], AF.Sqrt, scale=1.0 / D, bias=eps_ln[:])
                    nc.vector.reciprocal(rt[:], rt[:])
                    nc.vector.tensor_scalar(u_sb[:, tt, :], h_sb[:, tt, :], rt[:], None, op0=OP.mult)
                    u_bf = wp.tile([128, D], mybir.dt.bfloat16, tag="u_bf")
                    nc.vector.tensor_copy(u_bf[:], u_sb[:, tt, :])
                    nc.sync.dma_start(u_loc[tt * 128:(tt + 1) * 128, :], u_bf[:])
                for dt in range(DT):
                    for tt in range(2):
                        pt = ps.tile([128, 128], F32, tag="tpu")
                        nc.tensor.transpose(pt[:], u_sb[:, tt, dt * 128:(dt + 1) * 128], ident[:])
                        nc.vector.tensor_scalar(uT[:, dt, tt * 128:(tt + 1) * 128], pt[:],
                                                ln2_col[:, dt:dt + 1], None, op0=OP.mult)
                        nc.vector.tensor_scalar(uTr[:, dt, tt * 128:(tt + 1) * 128], pt[:],
                                                ln2_col[:, dt:dt + 1], None, op0=OP.mult)
            actp.release()

            # ---------- PHASE 3: router logits (fp32), AGs ----------
            with (
                tc.tile_pool(name="rt", bufs=2) as wp,
                tc.tile_pool(name="rtps", bufs=2, space="PSUM") as ps,
            ):
                wr_sb = wp.tile([128, DT, E], F32, tag="wr")
                nc.sync.dma_start(wr_sb[:], w_router[:])
                lg_sb = wp.tile([128, 2, E], F32, tag="lg")
                for tt in range(2):
                    lp = ps.tile([128, E], F32, tag="lgp")
                    for dt in range(DT):
                        nc.tensor.matmul(lp[:], uT[:, dt, tt * 128:(tt + 1) * 128],
                                         wr_sb[:, dt, :], start=(dt == 0), stop=(dt == DT - 1))
                    nc.vector.tensor_copy(lg_sb[:, tt, :], lp[:])
                    nc.sync.dma_start(lg_loc[tt * 128:(tt + 1) * 128, :], lg_sb[:, tt, :])
                nc.gpsimd.collective_compute("AllGather", OP.bypass, replica_groups=RG,
                                             ins=[lg_loc[:]], outs=[lg_ag[:]])
                nc.gpsimd.collective_compute("AllGather", OP.bypass, replica_groups=RG,
                                             ins=[u_loc[:]], outs=[u_ag[:]])
                nc.sync.dma_start(dbg_logits[:], lg_ag[:])

            # ---------- shared expert (natural layout, split for AG overlap) ----
            shp = tc.alloc_tile_pool(name="shp", bufs=1)
            hsT = shp.tile([128, MT, TPC], F32R, tag="hsT")
            shared_nat = shp.tile([128, 2, D], F32, tag="shared_nat")
            SCH = [(0, 512), (512, 1024), (1024, MS)]

            def shared_gate_up(tt):
                with (
                    tc.tile_pool(name=f"shA{tt}", bufs=2) as wp,
                    tc.tile_pool(name=f"shAw{tt}", bufs=3) as wgt,
                    tc.tile_pool(name=f"shAps{tt}", bufs=1, space="PSUM") as ps,
                ):
                    wgs_r = w_gs.rearrange("(dt p) m -> dt p m", p=128)
                    wus_r = w_us.rearrange("(dt p) m -> dt p m", p=128)
                    gp = [ps.tile([128, c1 - c0], F32, tag=f"sgp{i}", name=f"sgp{i}")
                          for i, (c0, c1) in enumerate(SCH)]
                    up = [ps.tile([128, c1 - c0], F32, tag=f"sup{i}", name=f"sup{i}")
                          for i, (c0, c1) in enumerate(SCH)]
                    for dt in range(DT):
                        wg = wgt.tile([128, MS], F32R, tag="wgs")
                        nc.sync.dma_start(wg[:], wgs_r[dt])
                        wu = wgt.tile([128, MS], F32R, tag="wus")
                        nc.sync.dma_start(wu[:], wus_r[dt])
                        for i, (c0, c1) in enumerate(SCH):
                            nc.tensor.matmul(gp[i][:], uTr[:, dt, tt * 128:(tt + 1) * 128],
                                             wg[:, c0:c1], start=(dt == 0), stop=(dt == DT - 1))
                            nc.tensor.matmul(up[i][:], uTr[:, dt, tt * 128:(tt + 1) * 128],
                                             wu[:, c0:c1], start=(dt == 0), stop=(dt == DT - 1))
                    hs_nat = wp.tile([128, MS], F32R, tag="hs_nat", bufs=1)
                    for i, (c0, c1) in enumerate(SCH):
                        sg = wp.tile([128, 512], F32, tag="sg")
                        nc.scalar.activation(sg[:, 0:c1 - c0], gp[i][:], AF.Silu)
                        nc.vector.tensor_tensor(hs_nat[:, c0:c1], sg[:, 0:c1 - c0], up[i][:],
                                                op=OP.mult)
                    for mt in range(MT):
                        tp = ps.tile([128, 128], F32R, tag="shtp", bufs=2)
                        nc.tensor.transpose(tp[:], hs_nat[:, mt * 128:(mt + 1) * 128], ident_r[:])
                        nc.vector.tensor_copy(hsT[:, mt, tt * 128:(tt + 1) * 128], tp[:])

            def shared_down():
                with (
                    tc.tile_pool(name="shB", bufs=2) as wp,
                    tc.tile_pool(name="shBw", bufs=3) as wgt,
                    tc.tile_pool(name="shBps", bufs=1, space="PSUM") as ps,
                ):
                    wds_r = w_ds.rearrange("(mt p) m -> mt p m", p=128)
                    yp = [[ps.tile([128, 512], F32, tag=f"syp{tt}{ch}", name=f"syp{tt}{ch}")
                           for ch in range(4)] for tt in range(2)]
                    for mt in range(MT):
                        wd = wgt.tile([128, D], F32R, tag="wds")
                        nc.sync.dma_start(wd[:], wds_r[mt])
                        for tt in range(2):
                            for ch in range(4):
                                nc.tensor.matmul(yp[tt][ch][:],
                                                 hsT[:, mt, tt * 128:(tt + 1) * 128],
                                                 wd[:, ch * 512:(ch + 1) * 512],
                                                 start=(mt == 0), stop=(mt == MT - 1))
                    for tt in range(2):
                        for ch in range(4):
                            nc.vector.tensor_copy(shared_nat[:, tt, ch * 512:(ch + 1) * 512],
                                                  yp[tt][ch][:])

            shared_gate_up(0)


            # ---------- PHASE 5: routing (redundant on all cores) ----------
            with (
                tc.tile_pool(name="rte", bufs=2) as wp,
                tc.tile_pool(name="rteps", bufs=1, space="PSUM") as ps,
            ):
                fi_sb = wp.tile([128, 16, 4], F32, tag="fi_sb", bufs=1)
                lg = wp.tile([128, 16, E], F32, tag="lgall")
                nc.sync.dma_start(lg[:], lg_ag.rearrange("(p g) e -> p g e", p=128))
                mx = wp.tile([128, 16], F32, tag="mx")
                nc.vector.tensor_reduce(mx[:], lg[:], axis=AX.X, op=OP.max)
                sh_ = wp.tile([128, 16, E], F32, tag="shift")
                nc.vector.tensor_tensor(sh_[:], lg[:], mx[:, :, None].to_broadcast((128, 16, E)),
                                        op=OP.subtract)
                ex = wp.tile([128, 16, E], F32, tag="ex")
                nc.scalar.activation(ex[:], sh_[:], AF.Exp)
                sm = wp.tile([128, 16], F32, tag="sm")
                nc.vector.tensor_reduce(sm[:], ex[:], axis=AX.X, op=OP.add)
                rs = wp.tile([128, 16], F32, tag="rs")
                nc.vector.reciprocal(rs[:], sm[:])
                probs = wp.tile([128, 16, E], F32, tag="probs")
                nc.vector.tensor_tensor(probs[:], ex[:], rs[:, :, None].to_broadcast((128, 16, E)),
                                        op=OP.mult)
                zt = wp.tile([128, 16], F32, tag="zt")
                nc.scalar.activation(zt[:], sm[:], AF.Ln)
                nc.vector.tensor_tensor(zt[:], zt[:], mx[:], op=OP.add)
                nc.vector.tensor_tensor(zt[:], zt[:], zt[:], op=OP.mult)
                z2r = wp.tile([128, 1], F32, tag="z2r")
                nc.vector.tensor_reduce(z2r[:], zt[:], axis=AX.X, op=OP.add)
                z2p = ps.tile([1, 1], F32, tag="z2p")
                nc.tensor.matmul(z2p[:], ones_col[:], z2r[:], start=True, stop=True)
                ind1 = wp.tile([128, 16, E], F32, tag="ind1")
                nc.vector.tensor_tensor(ind1[:], lg[:], mx[:, :, None].to_broadcast((128, 16, E)),
                                        op=OP.is_ge)
                mp1 = wp.tile([128, 16], F32, tag="mp1")
                nc.vector.tensor_reduce(mp1[:], probs[:], axis=AX.X, op=OP.max)
                p2 = wp.tile([128, 16, E], F32, tag="p2")
                nc.vector.tensor_tensor(p2[:], ind1[:], probs[:], op=OP.mult)
                nc.vector.tensor_tensor(p2[:], probs[:], p2[:], op=OP.subtract)
                mp2 = wp.tile([128, 16], F32, tag="mp2")
                nc.vector.tensor_reduce(mp2[:], p2[:], axis=AX.X, op=OP.max)
                ind2 = wp.tile([128, 16, E], F32, tag="ind2")
                nc.vector.tensor_tensor(ind2[:], p2[:], mp2[:, :, None].to_broadcast((128, 16, E)),
                                        op=OP.is_ge)
                ind = wp.tile([128, 16, E], F32, tag="ind")
                nc.vector.tensor_tensor(ind[:], ind1[:], ind2[:], op=OP.add)
                wsum = wp.tile([128, 16], F32, tag="wsum")
                nc.vector.tensor_tensor(wsum[:], mp1[:], mp2[:], op=OP.add)
                nc.vector.reciprocal(wsum[:], wsum[:])
                w1 = wp.tile([128, 16], F32, tag="w1")
                w2 = wp.tile([128, 16], F32, tag="w2")
                nc.vector.tensor_tensor(w1[:], mp1[:], wsum[:], op=OP.mult)
                nc.vector.tensor_tensor(w2[:], mp2[:], wsum[:], op=OP.mult)
                totals = wp.tile([128, E], F32, tag="totals")
                nc.vector.tensor_reduce(totals[:], ind[:].rearrange("p g e -> p e g"),
                                        axis=AX.X, op=OP.add)
                probsum = wp.tile([128, E], F32, tag="probsum")
                nc.vector.tensor_reduce(probsum[:], probs[:].rearrange("p g e -> p e g"),
                                        axis=AX.X, op=OP.add)
                c0 = wp.tile([128, 16, E], F32, tag="c0")
                c1 = wp.tile([128, 16, E], F32, tag="c1")
                nc.vector.tensor_copy(c0[:], ind[:])
                src, dstc = c0, c1
                for shf in (1, 2, 4, 8):
                    nc.vector.tensor_copy(dstc[:, :shf, :], src[:, :shf, :])
                    nc.vector.tensor_tensor(dstc[:, shf:, :], src[:, shf:, :],
                                            src[:, :16 - shf, :], op=OP.add)
                    src, dstc = dstc, src
                incl = src
                excl = wp.tile([128, 16, E], F32, tag="excl")
                nc.vector.tensor_tensor(excl[:], incl[:], ind[:], op=OP.subtract)
                offp = ps.tile([128, E], F32, tag="offp")
                nc.tensor.matmul(offp[:], triu_sb[:], totals[:], start=True, stop=True)
                offs = wp.tile([128, E], F32, tag="offs")
                nc.vector.tensor_copy(offs[:], offp[:])
                pos = wp.tile([128, 16, E], F32, tag="pos")
                nc.vector.tensor_tensor(pos[:], excl[:], offs[:, None, :].to_broadcast((128, 16, E)),
                                        op=OP.add)
                keep = wp.tile([128, 16, E], F32, tag="keep")
                nc.vector.tensor_scalar(keep[:], pos[:], float(CAP), None, op0=OP.is_lt)
                indk = wp.tile([128, 16, E], F32, tag="indk")
                nc.vector.tensor_tensor(indk[:], ind[:], keep[:], op=OP.mult)
                mpos = wp.tile([128, 16, E], F32, tag="mpos")
                nc.vector.tensor_tensor(mpos[:], pos[:], indk[:], op=OP.mult)
                t9 = wp.tile([128, 16, E], F32, tag="t9")
                nc.vector.tensor_scalar(t9[:], indk[:], -9999.0, 9999.0, op0=OP.mult, op1=OP.add)
                nc.vector.tensor_tensor(mpos[:], mpos[:], t9[:], op=OP.add)
                me = wp.tile([128, 16, E], F32, tag="me")
                nc.vector.tensor_tensor(me[:], mpos[:],
                                        onehot_sb[:, None, :].to_broadcast((128, 16, E)), op=OP.mult)
                msl = wp.tile([128, 16], F32, tag="msl")
                nc.vector.tensor_reduce(msl[:], me[:], axis=AX.X, op=OP.add)
                ip1 = ps.tile([1, 512], F32, tag="ip1")
                ip2 = ps.tile([1, CAP - 512], F32, tag="ip2")
                for g in range(16):
                    tval = wp.tile([128, 1], F32, tag="tval")
                    nc.vector.tensor_scalar(tval[:], p16_sb[:], float(g), None, op0=OP.add)
                    eq = wp.tile([128, CAP], F32, tag="eq")
                    nc.vector.tensor_scalar(eq[:], slot_iota_sb[:], msl[:, g:g + 1], None,
                                            op0=OP.is_equal)
                    nc.tensor.matmul(ip1[:], tval[:], eq[:, 0:512], start=(g == 0), stop=(g == 15))
                    nc.tensor.matmul(ip2[:], tval[:], eq[:, 512:CAP], start=(g == 0), stop=(g == 15))
                idx_f = wp.tile([1, CAP], F32, tag="idx_f")
                nc.vector.tensor_copy(idx_f[:, 0:512], ip1[:])
                nc.vector.tensor_copy(idx_f[:, 512:CAP], ip2[:])
                idx_i = wp.tile([1, CAP], I32, tag="idx_i")
                nc.vector.tensor_copy(idx_i[:], idx_f[:])
                nc.sync.dma_start(idx_dram[None, :], idx_i[:])

                def build_fi(indx, wx, col_f, col_w):
                    ec = wp.tile([128, 16, E], F32, tag="ec")
                    nc.vector.tensor_tensor(ec[:], indx[:],
                                            eiota_sb[:, None, :].to_broadcast((128, 16, E)), op=OP.mult)
                    ev = wp.tile([128, 16], F32, tag="ev")
                    nc.vector.tensor_reduce(ev[:], ec[:], axis=AX.X, op=OP.add)
                    sc = wp.tile([128, 16, E], F32, tag="sc")
                    nc.vector.tensor_tensor(sc[:], indx[:], pos[:], op=OP.mult)
                    sv = wp.tile([128, 16], F32, tag="sv")
                    nc.vector.tensor_reduce(sv[:], sc[:], axis=AX.X, op=OP.add)
                    kc = wp.tile([128, 16, E], F32, tag="kc")
                    nc.vector.tensor_tensor(kc[:], indx[:], keep[:], op=OP.mult)
                    kv_ = wp.tile([128, 16], F32, tag="kv_")
                    nc.vector.tensor_reduce(kv_[:], kc[:], axis=AX.X, op=OP.add)
                    fl = wp.tile([128, 16], F32, tag="fl")
                    nc.vector.tensor_scalar(fl[:], ev[:], float(CAP), None, op0=OP.mult)
                    nc.vector.tensor_tensor(fl[:], fl[:], sv[:], op=OP.add)
                    nc.vector.tensor_scalar(fl[:], fl[:], float(E * CAP - 1), None, op0=OP.min)
                    nc.vector.tensor_copy(fi_sb[:, :, col_f], fl[:])
                    wv_t = wp.tile([128, 16], F32, tag="wv_t")
                    nc.vector.tensor_tensor(wv_t[:], wx[:], kv_[:], op=OP.mult)
                    nc.vector.tensor_copy(fi_sb[:, :, col_w], wv_t[:])

                build_fi(ind1, w1, 0, 1)
                build_fi(ind2, w2, 2, 3)
                nc.sync.dma_start(fi_dram.rearrange("(p g) c -> p (g c)", p=128),
                                  fi_sb[:].rearrange("p g c -> p (g c)"))
                cnt_p = ps.tile([1, E], F32, tag="cntp")
                nc.tensor.matmul(cnt_p[:], ones_col[:], totals[:], start=True, stop=True)
                ps_p = ps.tile([1, E], F32, tag="psp")
                nc.tensor.matmul(ps_p[:], ones_col[:], probsum[:], start=True, stop=True)
                st = wp.tile([1, 32], F32, tag="stats")
                nc.vector.memset(st[:], 0.0)
                nc.vector.tensor_copy(st[:, 0:E], cnt_p[:])
                nc.vector.tensor_copy(st[:, 8:8 + E], ps_p[:])
                nc.vector.tensor_copy(st[:, 16:17], z2p[:])
                nc.sync.dma_start(stats_out[:], st[:])

            # combine metadata gather (ready as soon as fi_dram is written)
            cbm = tc.alloc_tile_pool(name="cbm", bufs=1)
            fi_my = cbm.tile([128, 2, 4], F32, tag="fi_my")
            of1 = cbm.tile([128, 2], I32, tag="of1")
            of2 = cbm.tile([128, 2], I32, tag="of2")
            for tt in range(2):
                nc.gpsimd.indirect_dma_start(
                    out=fi_my[:, tt, :], out_offset=None, in_=fi_dram[:],
                    in_offset=IndirectOffsetOnAxis(ap=own_rows_sb[:, tt:tt + 1], axis=0))
            nc.vector.tensor_copy(of1[:], fi_my[:, :, 0])
            nc.vector.tensor_copy(of2[:], fi_my[:, :, 2])

            # ---------- PHASE 6: expert FFN ----------
            with (
                tc.tile_pool(name="ex", bufs=2) as wp,
                tc.tile_pool(name="exps", bufs=2, space="PSUM") as ps,
                tc.tile_pool(name="exps_t", bufs=2, space="PSUM") as ps_t,
                tc.tile_pool(name="exps_y", bufs=1, space="PSUM") as ps_y,
            ):
                idx2 = wp.tile([128, 5], I32, tag="idx2")
                nc.sync.dma_start(idx2[:], idx_dram.rearrange("(g p) -> p g", p=128))
                exA = tc.alloc_tile_pool(name="exA", bufs=2)
                xbT = exA.tile([128, DT, CAP], F32R, tag="xbT", bufs=1)
                for gi in range(5):
                    xb_nat = exA.tile([128, D], mybir.dt.bfloat16, tag="xb_nat")
                    nc.gpsimd.indirect_dma_start(
                        out=xb_nat[:], out_offset=None, in_=u_ag[:],
                        in_offset=IndirectOffsetOnAxis(ap=idx2[:, gi:gi + 1], axis=0))
                    for dt in range(DT):
                        tp = ps_t.tile([128, 128], mybir.dt.bfloat16, tag="extp", bufs=1)
                        nc.tensor.transpose(tp[:], xb_nat[:, dt * 128:(dt + 1) * 128], ident_b[:])
                        nc.vector.tensor_scalar(xbT[:, dt, gi * 128:(gi + 1) * 128], tp[:],
                                                ln2_col[:, dt:dt + 1], None, op0=OP.mult)
                heT = wp.tile([128, MT, CAP], F32R, tag="heT", bufs=1)
                for mt in range(MT):
                    wg = exA.tile([128, DT, 128], F32R, tag="wge", bufs=2)
                    nc.sync.dma_start(wg[:], wge_h[mt])
                    wu = exA.tile([128, DT, 128], F32R, tag="wue", bufs=2)
                    nc.sync.dma_start(wu[:], wue_h[mt])
                    for ch in range(2):
                        cs = slice(ch * 320, (ch + 1) * 320)
                        gp = ps.tile([128, 320], F32, tag="gep")
                        up = ps.tile([128, 320], F32, tag="uep")
                        for dt in range(DT):
                            nc.tensor.matmul(gp[:], wg[:, dt, :], xbT[:, dt, cs],
                                             start=(dt == 0), stop=(dt == DT - 1))
                            nc.tensor.matmul(up[:], wu[:, dt, :], xbT[:, dt, cs],
                                             start=(dt == 0), stop=(dt == DT - 1))
                        sg = wp.tile([128, 320], F32, tag="sge")
                        nc.scalar.activation(sg[:], gp[:], AF.Silu)
                        nc.vector.tensor_tensor(heT[:, mt, cs], sg[:], up[:], op=OP.mult)
                exA.release()
                exB = tc.alloc_tile_pool(name="exB", bufs=2)
                yb_nat = exB.tile([128, 5, D], mybir.dt.bfloat16, tag="yb_nat", bufs=1)
                for dt in range(DT):
                    wd = exB.tile([128, MT, 128], F32R, tag="wde", bufs=2)
                    nc.sync.dma_start(wd[:], wde_h[dt])
                    ypA = ps_y.tile([128, 320], F32, tag="ydpA")
                    ypB = ps_y.tile([128, 320], F32, tag="ydpB")
                    for mt in range(MT):
                        nc.tensor.matmul(ypA[:], wd[:, mt, :], heT[:, mt, 0:320],
                                         start=(mt == 0), stop=(mt == MT - 1))
                        nc.tensor.matmul(ypB[:], wd[:, mt, :], heT[:, mt, 320:CAP],
                                         start=(mt == 0), stop=(mt == MT - 1))
                    ys = exB.tile([128, CAP], F32R, tag="ys")
                    nc.vector.tensor_copy(ys[:, 0:320], ypA[:])
                    nc.vector.tensor_copy(ys[:, 320:CAP], ypB[:])
                    for gi in range(5):
                        tp = ps_t.tile([128, 128], F32R, tag="extpr", bufs=1)
                        nc.tensor.transpose(tp[:], ys[:, gi * 128:(gi + 1) * 128], ident_r[:])
                        nc.vector.tensor_copy(yb_nat[:, gi, dt * 128:(dt + 1) * 128], tp[:])
                for gi in range(5):
                    nc.sync.dma_start(yb_loc[gi * 128:(gi + 1) * 128, :], yb_nat[:, gi, :])
                nc.gpsimd.collective_compute("AllGather", OP.bypass, replica_groups=RG,
                                             ins=[yb_loc[:]], outs=[yb_ag[:]])
                exB.release()

            # (shared part B emitted after expert AG)
            shared_gate_up(1)
            shared_down()

            # ---------- PHASE 7: combine ----------
            with tc.tile_pool(name="cb", bufs=2) as wp:
                for tt in range(2):
                    g1 = wp.tile([128, D], mybir.dt.bfloat16, tag="g1")
                    g2 = wp.tile([128, D], mybir.dt.bfloat16, tag="g2")
                    nc.gpsimd.indirect_dma_start(
                        out=g1[:], out_offset=None, in_=yb_ag[:],
                        in_offset=IndirectOffsetOnAxis(ap=of1[:, tt:tt + 1], axis=0))
                    nc.gpsimd.indirect_dma_start(
                        out=g2[:], out_offset=None, in_=yb_ag[:],
                        in_offset=IndirectOffsetOnAxis(ap=of2[:, tt:tt + 1], axis=0))
                    moe = wp.tile([128, D], F32, tag="moe")
                    moe2 = wp.tile([128, D], F32, tag="moe2")
                    nc.vector.tensor_scalar(moe[:], g1[:], fi_my[:, tt, 1:2], None, op0=OP.mult)
                    nc.vector.tensor_scalar(moe2[:], g2[:], fi_my[:, tt, 3:4], None, op0=OP.mult)
                    nc.vector.tensor_tensor(moe[:], moe[:], moe2[:], op=OP.add)
                    yt = wp.tile([128, D], F32, tag="yt")
                    nc.vector.tensor_tensor(yt[:], h_sb[:, tt, :], shared_nat[:, tt, :], op=OP.add)
                    nc.vector.tensor_tensor(yt[:], yt[:], moe[:], op=OP.add)
                    nc.sync.dma_start(y_out[tt * 128:(tt + 1) * 128, :], yt[:])
            cbm.release()
            shp.release()
            uTp.release()
            uTrp.release()

    nc.compile()
    return nc


_NC_CACHE = None


def _get_program():
    global _NC_CACHE
    if _NC_CACHE is None:
        _NC_CACHE = build_program()
    return _NC_CACHE


def _host_tables():
    inv_freq = 1.0 / THETA ** (np.arange(HALF, dtype=np.float32) / HALF)
    ang = np.arange(S, dtype=np.float32)[:, None] * inv_freq[None, :]
    cos_full = np.concatenate([np.cos(ang).T, np.cos(ang).T]).astype(np.float32)   # [64, S]
    ssin_full = np.concatenate([-np.sin(ang).T, np.sin(ang).T]).astype(np.float32)
    perm_to_true = np.empty(T, dtype=np.int64)
    for s in range(NB):
        tb = true_block(s)
        perm_to_true[s * 128:(s + 1) * 128] = np.arange(tb * 128, tb * 128 + 128)
    triu = np.triu(np.ones((128, 128), np.float32), 1)
    # swap64[k, m] = 1 iff m = (k+32) % 64  (out = swap64.T @ x swaps halves)
    swap64 = np.zeros((ROT, ROT), np.float32)
    for k_ in range(ROT):
        swap64[k_, (k_ + HALF) % ROT] = 1.0
    slot_iota = np.broadcast_to(np.arange(CAP, dtype=np.float32), (128, CAP)).copy()
    p16 = (np.arange(128, dtype=np.float32) * 16).reshape(128, 1)
    e_iota = np.broadcast_to(np.arange(E, dtype=np.float32), (128, E)).copy()
    kpos_cols = np.empty((128, NB), np.float32)
    for s in range(NB):
        kpos_cols[:, s] = perm_to_true[s * 128:(s + 1) * 128]
    return cos_full, ssin_full, perm_to_true, triu, slot_iota, p16, e_iota, kpos_cols, swap64


def _col_major(w, n_outer, q=128):
    """[D_in, n_outer*q] -> [n_outer, 128, D_in//128, q] with
    (o, p, t, c) = w[t*128+p, o*q+c]; contiguous per (o, p)."""
    d_in = w.shape[0]
    return np.ascontiguousarray(
        w.reshape(d_in // 128, 128, n_outer, q).transpose(2, 1, 0, 3))


def kernel(x, ln1_w, ln2_w, w_q, w_k, w_v, w_o, attn_gate, w_router,
           w_gate_e, w_up_e, w_down_e, w_gate_s, w_up_s, w_down_s,
           _trace=False):
    nc = _get_program()
    (cos_full, ssin_full, perm_to_true, triu, slot_iota, p16, e_iota, kpos_cols,
     swap64) = _host_tables()

    f32 = lambda a: np.ascontiguousarray(np.asarray(a, dtype=np.float32))
    x2 = f32(x).reshape(T, D)
    w_gate_e, w_up_e, w_down_e = f32(w_gate_e), f32(w_up_e), f32(w_down_e)
    shared_inputs = dict(
        x_nat=None,
        w_q=f32(w_q), w_k=f32(w_k),
        w_v=f32(w_v), w_o=f32(w_o),
        attn_gate=f32(attn_gate),
        wr_h=np.ascontiguousarray(f32(w_router).reshape(DT, 128, E).transpose(1, 0, 2)),
        w_gs=f32(w_gate_s), w_us=f32(w_up_s), w_ds=f32(w_down_s),
        ln1_w=f32(ln1_w), ln2_w=f32(ln2_w),
        triu=triu, slot_iota=slot_iota, p16_col=p16, e_iota=e_iota,
        kpos_cols=kpos_cols, swap64_t=swap64,
    )
    in_maps = []
    for c in range(NC):
        rows = perm_to_true[c * TPC:(c + 1) * TPC]
        m = dict(shared_inputs)
        m["x_nat"] = np.ascontiguousarray(x2[rows])
        m["wge_h"] = _col_major(w_gate_e[c], MT)
        m["wue_h"] = _col_major(w_up_e[c], MT)
        m["wde_h"] = _col_major(w_down_e[c], DT)
        m["cos2_t"] = np.ascontiguousarray(cos_full[:, rows])
        m["ssin2_t"] = np.ascontiguousarray(ssin_full[:, rows])
        m["qpos_bc"] = np.ascontiguousarray(
            np.broadcast_to(rows.astype(np.float32), (128, TPC)))
        m["onehot_e"] = np.broadcast_to(
            (np.arange(E) == c).astype(np.float32), (128, E)).copy()
        m["own_rows"] = np.ascontiguousarray(
            (c * TPC + np.arange(TPC, dtype=np.int32)).reshape(2, 128).T)
        in_maps.append(m)

    res = run_bass_kernel_spmd(nc, in_maps, core_ids=list(range(NC)), trace=_trace)

    y = np.empty((T, D), np.float32)
    for c in range(NC):
        rows = perm_to_true[c * TPC:(c + 1) * TPC]
        y[rows] = res.results[c]["y_out"]
    st = res.results[0]["stats_out"][0]
    counts, probsum, z2 = st[0:E], st[8:8 + E], st[16]
    frac = counts / max(float(counts.sum()), 1.0)
    mean_probs = probsum / T
    lb_loss = E * float((frac * K * mean_probs).sum())
    z_loss = float(z2) / T
    aux = np.float32(0.01 * lb_loss + 0.001 * z_loss)
    out = (y.reshape(B, S, D), aux)
    if _trace:
        return out, res
    return out
